# revision 47
# baseline (speedup 1.0000x reference)
"""Trainium2 Bass kernel for nn_AdvectionDiffusionReaction2M (v3).

Advection-diffusion-reaction on a 512x512 grid, 199 sequential steps, output =
all intermediate states (199,512,512) f32.

Sharding: rows split 8 ways (64 rows/core) with 16-row ghost zones refreshed
by an AllGather every 16 steps.  SBUF layout per core: flat [128, 6B+2] f32
per state buffer:
    [ GL (B) | pad | b1 b2 b3 b4 (4B) | pad | GR (B) ]
partition p = column group (cols 4p..4p+3 at blocks b1..b4), GL/GR = ghost
columns 4p-1 / 4p+4, i = stored row (96 = 16 ghost + 64 + 16 ghost).  The two
pad columns make the Up/Dn offset views disjoint from GL/GR, so the ghost
column refresh (PE partition-shift matmuls + PSUM->SBUF copies) overlaps the
next step's leading DVE ops instead of serializing the whole step.

The update is regrouped per neighbor with Tc-dependent coefficients
   Tn = Up*(s+h*Tc^2) + Dn*(s-h*Tc^2) + L*(s-h*Tc) + R*(s+h*Tc) + phi(Tc)
   phi = Tc + g*(Tc^3-Tc^2+Tc),  g = h*2dx
computed by fused custom DVE ops (block-edge rows are sacrificial ghost rows,
so row-crossing garbage in Up/Dn is harmless).  L and R are split into an
interior part (blocks) and a one-block ghost part (GL/GR) per pass.
"""

import os
import numpy as np

N = 512
DX = 1.0 / (N - 1)
DT = 1e-7
MB = 256
NCORES = 8
K = 16                      # ghost depth (rows)
RS = 64 + 2 * K             # stored rows per core (96)
NSTEPS = int(os.environ.get("ADR_NSTEPS", "199"))
B = RS                      # block stride in flat free dim
F = 6 * B + 2               # flat state width: GL|pad|b1..b4|pad|GR
S0 = B + 1                  # flat offset of block 1 (state region start)

LAST_EXEC_NS = None
LAST_RESULT = None

_OPS_REGISTERED = {}


def _register_ops():
    """Register custom DVE ops (runtime registration into dve_ops.OPS)."""
    if _OPS_REGISTERED:
        return _OPS_REGISTERED
    import concourse.dve_ops as dve_ops
    from concourse.dve_ops import DveOp, OPS
    from concourse.dve_spec import Spec, Src0, Src1, C0, C1, C2, One, sq, lower
    from concourse.dve_uop import DveOpSpec

    def make_op(name, body, reference):
        for op in OPS:
            if op.name == name:
                return op
        spec = Spec(body=body, reference=reference)
        shas = {}
        for ver in ("v3", "v4"):
            uops = lower(spec, ver=ver)
            tmp = DveOpSpec(name=name, opcode=0, uops=uops, rd1_en=True)
            shas[ver] = tmp.sha(ver)
        op = DveOp(name, spec, subdim=False, uops_sha=shas)
        OPS.append(op)
        dve_ops._SUB_OPCODE_FOR_NAME[name] = (
            dve_ops._CUSTOM_DVE_ROW_BASE + len(OPS) - 1)
        assert dve_ops._SUB_OPCODE_FOR_NAME[name] < 0x20, "opcode row overflow"
        dve_ops.CUSTOM_DVE_SPECS[name] = spec
        return op

    q = sq(Src0)
    gc = C0 * C2                          # g = h * 2dx (hoisted mult)
    # out = Up*(s + h*Tc^2) + g*(Tc^2 - Tc)*Tc      [phi part 1: g(Tc^3-Tc^2)]
    _OPS_REGISTERED["APHI"] = make_op(
        "ADR_APHI",
        Src1 * (C1 + q * C0) + (q - Src0) * gc * Src0,
        lambda in0, in1, s0, s1, imm2:
            in1 * (s1 + in0**2 * s0)
            + (in0**2 - in0) * (s0 * imm2) * in0)
    # out = Dn*(s - h*Tc^2)
    _OPS_REGISTERED["BSQ"] = make_op(
        "ADR_BSQ", Src1 * (C1 - q * C0),
        lambda in0, in1, s0, s1: in1 * (s1 - in0**2 * s0))
    # out = L*(s - h*Tc) + (h*Tc)*2dx               [phi part 3: g*Tc]
    _a = Src0 * C0
    _OPS_REGISTERED["CLIN"] = make_op(
        "ADR_CLIN", Src1 * (C1 - _a) + _a * C2,
        lambda in0, in1, s0, s1, imm2:
            in1 * (s1 - in0 * s0) + in0 * s0 * imm2)
    # out = R*(s + h*Tc) + Tc + (-4)*s*Tc           [phi part 2: (1-4s)Tc]
    _OPS_REGISTERED["DLIN"] = make_op(
        "ADR_DLIN", Src1 * (C1 + _a) + Src0 + Src0 * C1 * C2,
        lambda in0, in1, s0, s1, imm2:
            in1 * (s1 + in0 * s0) + in0 + in0 * s1 * imm2)
    # out = Src0*C0 + Src1*C1  (masked blend / select)
    _OPS_REGISTERED["SEL"] = make_op(
        "ADR_SEL", Src0 * C0 + Src1 * C1,
        lambda in0, in1, s0, s1: in0 * s0 + in1 * s1)
    return _OPS_REGISTERED


def _pack_core(G, c):
    """Full grid (512,512) -> per-core flat tile [128, F] (f32, zero padded).

    Layout per partition p: [GL | 0 | b1 b2 b3 b4 | 0 | GR] where block bj
    holds column 4p+j-1 over the RS stored rows and GL/GR hold cols 4p-1 /
    4p+4.
    """
    lo = 64 * c - K
    S = np.zeros((RS, N), np.float32)
    g0, g1 = max(lo, 0), min(lo + RS, N)
    S[g0 - lo: g1 - lo] = G[g0:g1]
    cols = (4 * np.arange(128)[:, None] - 1 + np.arange(6)[None, :])  # [128,6]
    valid = (cols >= 0) & (cols < N)
    t = S.T[np.clip(cols, 0, N - 1)]          # [128, 6, RS]
    t[~valid] = 0.0
    flat = np.zeros((128, F), np.float32)
    flat[:, 0:B] = t[:, 0]                      # GL
    flat[:, S0:S0 + 4 * B] = t[:, 1:5].reshape(128, 4 * B)
    flat[:, 5 * B + 2:6 * B + 2] = t[:, 5]      # GR
    return np.ascontiguousarray(flat, dtype=np.float32)


def _build(nc, tile, mybir, bass, scal):
    f32 = mybir.dt.float32
    u32 = mybir.dt.uint32
    OP = mybir.AluOpType
    ops = _register_ops()
    APHI, BSQ, CLIN, DLIN, SEL = (ops[k] for k in
                                  ("APHI", "BSQ", "CLIN", "DLIN", "SEL"))

    bf16 = mybir.dt.bfloat16
    BF = bf16 if os.environ.get("ADR_BF16", "1") == "1" else f32
    AF = mybir.ActivationFunctionType
    u0s_d = nc.dram_tensor("u0s", [128, F], f32, kind="ExternalInput").ap()
    ppc_d = nc.dram_tensor("ppc", [128, 16], f32, kind="ExternalInput").ap()
    wr_d = nc.dram_tensor("wr", [128, 128], f32, kind="ExternalInput").ap()
    wl_d = nc.dram_tensor("wl", [128, 128], f32, kind="ExternalInput").ap()
    nbrs_d = nc.dram_tensor("nbrs", [1, 2], u32, kind="ExternalInput").ap()
    rsel_d = nc.dram_tensor("rsel", [1, 2], u32, kind="ExternalInput").ap()
    msk_d = nc.dram_tensor("msk", [128, 2 * B], f32, kind="ExternalInput").ap()
    out_d = nc.dram_tensor("out", [NSTEPS, 128, 4, 64], f32,
                           kind="ExternalOutput").ap()

    ca, cb = scal["ca"], scal["cb"]

    # ghost sync every K steps (synchronous: state-t bands must merge into the
    # state-t tile before step t+1 — any lag breaks time-consistency)
    nsync = [t for t in range(K, NSTEPS, K)]

    with tile.TileContext(nc) as tc:
        with tc.tile_pool(name="state", bufs=1) as sp, \
             tc.tile_pool(name="tmp", bufs=2) as tp, \
             tc.tile_pool(name="psum", bufs=2, space="PSUM") as pp, \
             tc.tile_pool(name="dram", bufs=1, space="DRAM") as dp:

            tA = sp.tile([128, F], f32, tag="tA")
            tB = sp.tile([128, F], f32, tag="tB")
            ppc = sp.tile([128, 16], f32, tag="ppc")
            msk = sp.tile([128, 2 * B], f32, tag="msk")
            wr = sp.tile([128, 128], f32, tag="wr")
            wl = sp.tile([128, 128], f32, tag="wl")

            cc_in = dp.tile([2, 128, 4, K], f32, tag="ccin")
            cc_outs = {t: dp.tile([16 * 128, 4, K], f32, tag=f"ccout{t}",
                                  name=f"ccout{t}", addr_space="Shared")
                       for t in nsync}
            warm_in = dp.tile([1, 64], f32, tag="warmin")
            warm_out = dp.tile([8, 64], f32, tag="warmout", name="warmout",
                               addr_space="Shared")

            nc.sync.dma_start(tA[:], u0s_d[:])
            nc.sync.dma_start(ppc[:], ppc_d[:])
            nc.sync.dma_start(msk[:], msk_d[:])
            nc.sync.dma_start(wr[:], wr_d[:])
            nc.sync.dma_start(wl[:], wl_d[:])

            rp = nc.alloc_registers("rprev")
            nc.regs_load(rp, nbrs_d[0:1, 0:1])
            sv_prev = nc.snap(rp, min_val=0, max_val=15 * 128)
            rn = nc.alloc_registers("rnext")
            nc.regs_load(rn, nbrs_d[0:1, 1:2])
            sv_next = nc.snap(rn, min_val=0, max_val=15 * 128)
            # per-core Neumann source rows (core 0: 17 else 16; core 7: 78 else 79)
            rt0 = nc.alloc_registers("rtop")
            nc.regs_load(rt0, rsel_d[0:1, 0:1])
            sv_rtop = nc.snap(rt0, min_val=K, max_val=K + 1)
            rb0 = nc.alloc_registers("rbot")
            nc.regs_load(rb0, rsel_d[0:1, 1:2])
            sv_rbot = nc.snap(rb0, min_val=K + 62, max_val=K + 63)

            s_ = ppc[:, 3:4]; h_ = ppc[:, 4:5]
            ifA = ppc[:, 9:10]; ifB = ppc[:, 10:11]; ifC = ppc[:, 11:12]
            cD = ppc[:, 12:13]; cE = ppc[:, 13:14]
            one_ = ppc[:, 14:15]

            # warm-up AllGather (tiny payload): pays the cold-start cost of
            # the CC engine while the first steps compute, so the step-16
            # collective runs at warm latency.
            nc.gpsimd.collective_compute(
                "AllGather", OP.bypass,
                replica_groups=[list(range(NCORES))],
                ins=[warm_in[:]], outs=[warm_out[:]])

            cur, nxt = tA, tB
            # ghost refresh for the initial state (u0 pack already provides
            # valid GL/GR, so nothing needed before step 1)
            for t in range(1, NSTEPS + 1):
                # views into the flat state
                Tc = cur[:, S0:S0 + 4 * B]
                Up = cur[:, S0 - 1:S0 + 4 * B - 1]
                Dn = cur[:, S0 + 1:S0 + 4 * B + 1]
                GLc = cur[:, 0:B]
                GRc = cur[:, 5 * B + 2:6 * B + 2]
                TnV = nxt[:, S0:S0 + 4 * B]

                PU = tp.tile([128, 4 * B], BF, tag="PU")
                PD = tp.tile([128, 4 * B], BF, tag="PD")
                PL = tp.tile([128, 4 * B], BF, tag="PL")
                PR = tp.tile([128, 4 * B], f32, tag="PR")
                S1 = tp.tile([128, 4 * B], BF, tag="S1")
                S2 = tp.tile([128, 4 * B], BF, tag="S2")
                I1 = tp.tile([128, B], f32, tag="I1")

                # interface precompute (reads OLD state only): one SEL
                # custom op at partition base 0 (custom scalar APs only work
                # at base 0): I1 = ifA*GRc + ifB*b3 (nonzero only at p63)
                nc.vector._custom_dve(
                    SEL, out=I1[:], in0=GRc,
                    in1=cur[:, S0 + 2 * B:S0 + 3 * B], s0=ifA, s1=ifB)

                # neighbor-grouped fused passes (DVE) + bf16 sum tree.
                # Ghost-column parts (CG/DG) are scheduled late so the ghost
                # refresh of the previous step can overlap the leading ops.
                nc.vector._custom_dve(APHI, out=PU[:], in0=Tc, in1=Up,
                                      s0=h_, s1=s_, imm2=2.0 * DX)
                nc.vector._custom_dve(BSQ, out=PD[:], in0=Tc, in1=Dn,
                                      s0=h_, s1=s_)
                # DLIN over blocks 1-3 (R = blocks 2-4), carries phi
                nc.vector._custom_dve(DLIN, out=PR[:, 0:3 * B],
                                      in0=cur[:, S0:S0 + 3 * B],
                                      in1=cur[:, S0 + B:S0 + 4 * B],
                                      s0=h_, s1=s_, imm2=-4.0)
                nc.vector.tensor_tensor(S1[:], PU[:], PD[:], OP.add)
                # CLIN over blocks 2-4 (L = blocks 1-3), carries phi part 3
                nc.vector._custom_dve(CLIN, out=PL[:, B:4 * B],
                                      in0=cur[:, S0 + B:S0 + 4 * B],
                                      in1=cur[:, S0:S0 + 3 * B],
                                      s0=h_, s1=s_, imm2=2.0 * DX)
                # ghost parts (same ops so phi lands exactly once per point):
                # block 1 L from GL, block 4 R from GR
                nc.vector._custom_dve(CLIN, out=PL[:, 0:B],
                                      in0=cur[:, S0:S0 + B], in1=GLc,
                                      s0=h_, s1=s_, imm2=2.0 * DX)
                nc.vector._custom_dve(DLIN, out=PR[:, 3 * B:4 * B],
                                      in0=cur[:, S0 + 3 * B:S0 + 4 * B],
                                      in1=GRc, s0=h_, s1=s_, imm2=-4.0)
                nc.vector.tensor_tensor(S2[:], S1[:], PL[:], OP.add)
                nc.vector.tensor_tensor(TnV, S2[:], PR[:], OP.add)

                # interface blend into b4: SEL in-place, full partitions
                # (ifC = 1 except p63=0; I1 nonzero only at p63)
                b4v = nxt[:, S0 + 3 * B:S0 + 4 * B]
                nc.vector._custom_dve(SEL, out=b4v, in0=b4v, in1=I1[:],
                                      s0=ifC, s1=one_)

                # row boundary (Neumann) with per-core dynamic source row
                # (middle cores self-copy).  Blocks 1-3 go on ACT right
                # after TnV; block 4 (which must wait for the interface
                # blend) goes on Vector so the ACT queue is never stalled.
                nx4 = nxt[:, S0:S0 + 4 * B].rearrange("p (b i) -> p b i", b=4)
                nc.scalar.copy(nx4[:, 0:3, K:K + 1],
                               nx4[:, 0:3, bass.ds(sv_rtop, 1)])
                nc.scalar.copy(nx4[:, 0:3, K + 63:K + 64],
                               nx4[:, 0:3, bass.ds(sv_rbot, 1)])

                # col 511 (p127) Neumann: SEL b4 <- cE*b4 + cD*b3 in-place,
                # full partitions.  Reads b3 rows post-row-copy; the block-4
                # row copies afterwards are idempotent at p127.
                nc.vector._custom_dve(
                    SEL, out=b4v, in0=b4v,
                    in1=nxt[:, S0 + 2 * B:S0 + 3 * B], s0=cE, s1=cD)
                nc.vector.tensor_scalar_add(
                    nx4[:, 3:4, K:K + 1], nx4[:, 3:4, bass.ds(sv_rtop, 1)],
                    0.0)
                nc.vector.tensor_scalar_add(
                    nx4[:, 3:4, K + 63:K + 64],
                    nx4[:, 3:4, bass.ds(sv_rbot, 1)], 0.0)

                # column boundary: col 0 (p0) on ACT
                nc.scalar.copy(nxt[0:1, S0:S0 + B], nxt[0:1, S0 + B:S0 + 2 * B])

                # ghost row sync (blocking; gpsimd queue keeps Sync free).
                # Bands carry only the 4 state blocks; GL/GR are rebuilt by
                # the ghost-column matmuls placed AFTER the merge below.
                if t in nsync:
                    cc_out = cc_outs[t]
                    nc.gpsimd.dma_start(cc_in[0], nx4[:, :, K:2 * K])
                    nc.gpsimd.dma_start(cc_in[1], nx4[:, :, 64:64 + K])
                    nc.gpsimd.collective_compute(
                        "AllGather", OP.bypass,
                        replica_groups=[list(range(NCORES))],
                        ins=[cc_in[:]], outs=[cc_out[:]])
                    nc.gpsimd.dma_start(nx4[:, :, 0:K],
                                        cc_out[bass.ds(sv_prev, 128)])
                    nc.gpsimd.dma_start(nx4[:, :, 64 + K:64 + 2 * K],
                                        cc_out[bass.ds(sv_next, 128)])

                # ghost column refresh via partition-shift matmuls (full
                # rows).  At sync steps this reads the merged tile, so the
                # new GL/GR include fresh ghost-row values.
                psR = pp.tile([128, B], f32, tag="psR")
                psL = pp.tile([128, B], f32, tag="psL")
                nc.tensor.matmul(psR[:], wr[:], nxt[:, S0:S0 + B],
                                 start=True, stop=True)
                nc.tensor.matmul(psL[:], wl[:], nxt[:, S0 + 3 * B:S0 + 4 * B],
                                 start=True, stop=True)
                nc.scalar.copy(nxt[:, 5 * B + 2:6 * B + 2], psR[:])
                nc.scalar.copy(nxt[:, 0:B], psL[:])

                # output: owned rows (the read has two steps of slack)
                nc.sync.dma_start(out_d[t - 1], nx4[:, :, K:K + 64])

                cur, nxt = nxt, cur
    return nc


def _ensure_ntff_hook():
    """Provide antenv.axon_hooks (missing in this image) so bass_utils can
    NTFF-profile under axon."""
    import sys
    import types
    try:
        from antenv.axon_hooks import get_axon_ntff_profile_hook  # noqa: F401
        return
    except ImportError:
        pass
    mod = types.ModuleType("antenv.axon_hooks")
    mod._hook = None

    def set_axon_ntff_profile_hook(h):
        mod._hook = h

    def get_axon_ntff_profile_hook():
        return mod._hook

    mod.set_axon_ntff_profile_hook = set_axon_ntff_profile_hook
    mod.get_axon_ntff_profile_hook = get_axon_ntff_profile_hook
    sys.modules["antenv.axon_hooks"] = mod
    import antenv
    antenv.axon_hooks = mod
    try:
        from trn_agent_boot.trn_boot import _ntff_profile_via_ctypes
        hook = _ntff_profile_via_ctypes("/opt/axon/libaxon_pjrt.so")
        if hook is not None:
            mod._hook = hook
    except Exception:
        pass


def kernel(u0, k1, k2, alpha1, alpha2):
    global LAST_EXEC_NS, LAST_RESULT
    import concourse.bacc as bacc
    import concourse.bass as bass
    import concourse.tile as tile
    import concourse.mybir as mybir
    from concourse.bass_utils import run_bass_kernel_spmd

    u0 = np.asarray(u0, dtype=np.float32)
    k1f = float(np.asarray(k1).reshape(-1)[0])
    k2f = float(np.asarray(k2).reshape(-1)[0])
    a1f = float(np.asarray(alpha1).reshape(-1)[0])
    a2f = float(np.asarray(alpha2).reshape(-1)[0])

    dx2 = DX * DX
    scal = {"ca": k1f / (k1f + k2f), "cb": k2f / (k1f + k2f)}

    nc = bacc.Bacc(
        "TRN2", target_bir_lowering=False, debug=False,
        num_devices=NCORES,
    )
    _build(nc, tile, mybir, bass, scal)
    nc.compile()

    left = np.arange(128) < 64
    s = np.where(left, DT * a1f / dx2, DT * a2f / dx2).astype(np.float32)
    h = np.where(left, DT * k1f / (2 * DX), DT * k2f / (2 * DX)).astype(np.float32)
    g = np.where(left, DT * k1f, DT * k2f).astype(np.float32)
    WR = np.eye(128, k=-1, dtype=np.float32)   # out[m] = in[m+1]
    WL = np.eye(128, k=+1, dtype=np.float32)   # out[m] = in[m-1]

    m63 = (np.arange(128) == 63).astype(np.float32)
    m127 = (np.arange(128) == 127).astype(np.float32)
    in_maps = []
    for c in range(NCORES):
        ppc = np.zeros((128, 16), np.float32)
        ppc[:, 0] = g
        ppc[:, 1] = -g
        ppc[:, 2] = 1.0 - 4.0 * s + g
        ppc[:, 3] = s
        ppc[:, 4] = h
        ppc[:, 9] = m63 * scal["ca"]       # ifA
        ppc[:, 10] = m63 * scal["cb"]      # ifB
        ppc[:, 11] = 1.0 - m63             # ifC
        ppc[:, 12] = m127                  # cD
        ppc[:, 13] = 1.0 - m127            # cE
        ppc[:, 14] = 1.0                   # ones (SEL s1 for interface)
        prev_off = (2 * (c - 1) + 1) * 128 if c > 0 else 0
        next_off = (2 * (c + 1)) * 128 if c < NCORES - 1 else 0
        rtop = K + 1 if c == 0 else K
        rbot = K + 62 if c == NCORES - 1 else K + 63
        in_maps.append({
            "u0s": _pack_core(u0, c),
            "ppc": ppc,
            "wr": WR,
            "wl": WL,
            "nbrs": np.array([[prev_off, next_off]], dtype=np.uint32),
            "rsel": np.array([[rtop, rbot]], dtype=np.uint32),
            "msk": np.concatenate([
                np.broadcast_to((m63 * scal["ca"])[:, None], (128, B)),
                np.broadcast_to((m63 * scal["cb"])[:, None], (128, B)),
            ], axis=1).astype(np.float32).copy(),
        })

    trace = os.environ.get("ADR_TRACE", "0") == "1"
    if trace:
        _ensure_ntff_hook()
    res = run_bass_kernel_spmd(
        nc, in_maps, core_ids=list(range(NCORES)), trace=trace)
    LAST_EXEC_NS = res.exec_time_ns
    LAST_RESULT = res

    full = np.zeros((NSTEPS, N, N), np.float32)
    for c in range(NCORES):
        arr = np.asarray(res.results[c]["out"]).reshape(NSTEPS, 128, 4, 64)
        full[:, 64 * c:64 * (c + 1), :] = (
            arr.transpose(0, 3, 1, 2).reshape(NSTEPS, 64, 512))
    return full


# revision 49
# speedup vs baseline: 1.1605x; 1.1605x over previous
"""Trainium2 Bass kernel for nn_AdvectionDiffusionReaction2M (v3).

Advection-diffusion-reaction on a 512x512 grid, 199 sequential steps, output =
all intermediate states (199,512,512) f32.

Sharding: rows split 8 ways (64 rows/core) with 16-row ghost zones refreshed
by an AllGather every 16 steps.  SBUF layout per core: flat [128, 6B+2] f32
per state buffer:
    [ GL (B) | pad | b1 b2 b3 b4 (4B) | pad | GR (B) ]
partition p = column group (cols 4p..4p+3 at blocks b1..b4), GL/GR = ghost
columns 4p-1 / 4p+4, i = stored row (96 = 16 ghost + 64 + 16 ghost).  The two
pad columns make the Up/Dn offset views disjoint from GL/GR, so the ghost
column refresh (PE partition-shift matmuls + PSUM->SBUF copies) overlaps the
next step's leading DVE ops instead of serializing the whole step.

The update is regrouped per neighbor with Tc-dependent coefficients
   Tn = Up*(s+h*Tc^2) + Dn*(s-h*Tc^2) + L*(s-h*Tc) + R*(s+h*Tc) + phi(Tc)
   phi = Tc + g*(Tc^3-Tc^2+Tc),  g = h*2dx
computed by fused custom DVE ops (block-edge rows are sacrificial ghost rows,
so row-crossing garbage in Up/Dn is harmless).  L and R are split into an
interior part (blocks) and a one-block ghost part (GL/GR) per pass.
"""

import os
import numpy as np

N = 512
DX = 1.0 / (N - 1)
DT = 1e-7
MB = 256
NCORES = 8
K = 16                      # ghost depth (rows)
RS = 64 + 2 * K             # stored rows per core (96)
NSTEPS = int(os.environ.get("ADR_NSTEPS", "199"))
B = RS                      # block stride in flat free dim
F = 6 * B + 2               # flat state width: GL|pad|b1..b4|pad|GR
S0 = B + 1                  # flat offset of block 1 (state region start)

LAST_EXEC_NS = None
LAST_RESULT = None

_OPS_REGISTERED = {}


def _register_ops():
    """Register custom DVE ops (runtime registration into dve_ops.OPS)."""
    if _OPS_REGISTERED:
        return _OPS_REGISTERED
    import concourse.dve_ops as dve_ops
    from concourse.dve_ops import DveOp, OPS
    from concourse.dve_spec import Spec, Src0, Src1, C0, C1, C2, One, sq, lower
    from concourse.dve_uop import DveOpSpec

    def make_op(name, body, reference):
        for op in OPS:
            if op.name == name:
                return op
        spec = Spec(body=body, reference=reference)
        shas = {}
        for ver in ("v3", "v4"):
            uops = lower(spec, ver=ver)
            tmp = DveOpSpec(name=name, opcode=0, uops=uops, rd1_en=True)
            shas[ver] = tmp.sha(ver)
        op = DveOp(name, spec, subdim=False, uops_sha=shas)
        OPS.append(op)
        dve_ops._SUB_OPCODE_FOR_NAME[name] = (
            dve_ops._CUSTOM_DVE_ROW_BASE + len(OPS) - 1)
        assert dve_ops._SUB_OPCODE_FOR_NAME[name] < 0x20, "opcode row overflow"
        dve_ops.CUSTOM_DVE_SPECS[name] = spec
        return op

    q = sq(Src0)
    gc = C0 * C2                          # g = h * 2dx (hoisted mult)
    # out = Up*(s + h*Tc^2) + g*(Tc^2 - Tc)*Tc      [phi part 1: g(Tc^3-Tc^2)]
    _OPS_REGISTERED["APHI"] = make_op(
        "ADR_APHI",
        Src1 * (C1 + q * C0) + (q - Src0) * gc * Src0,
        lambda in0, in1, s0, s1, imm2:
            in1 * (s1 + in0**2 * s0)
            + (in0**2 - in0) * (s0 * imm2) * in0)
    # out = Dn*(s - h*Tc^2)
    _OPS_REGISTERED["BSQ"] = make_op(
        "ADR_BSQ", Src1 * (C1 - q * C0),
        lambda in0, in1, s0, s1: in1 * (s1 - in0**2 * s0))
    # out = L*(s - h*Tc) + (h*Tc)*2dx               [phi part 3: g*Tc]
    _a = Src0 * C0
    _OPS_REGISTERED["CLIN"] = make_op(
        "ADR_CLIN", Src1 * (C1 - _a) + _a * C2,
        lambda in0, in1, s0, s1, imm2:
            in1 * (s1 - in0 * s0) + in0 * s0 * imm2)
    # out = R*(s + h*Tc) + Tc + (-4)*s*Tc           [phi part 2: (1-4s)Tc]
    _OPS_REGISTERED["DLIN"] = make_op(
        "ADR_DLIN", Src1 * (C1 + _a) + Src0 + Src0 * C1 * C2,
        lambda in0, in1, s0, s1, imm2:
            in1 * (s1 + in0 * s0) + in0 + in0 * s1 * imm2)
    # out = Src0*C0 + Src1*C1  (masked blend / select)
    _OPS_REGISTERED["SEL"] = make_op(
        "ADR_SEL", Src0 * C0 + Src1 * C1,
        lambda in0, in1, s0, s1: in0 * s0 + in1 * s1)
    return _OPS_REGISTERED


def _pack_core(G, c):
    """Full grid (512,512) -> per-core flat tile [128, F] (f32, zero padded).

    Layout per partition p: [GL | 0 | b1 b2 b3 b4 | 0 | GR] where block bj
    holds column 4p+j-1 over the RS stored rows and GL/GR hold cols 4p-1 /
    4p+4.
    """
    lo = 64 * c - K
    S = np.zeros((RS, N), np.float32)
    g0, g1 = max(lo, 0), min(lo + RS, N)
    S[g0 - lo: g1 - lo] = G[g0:g1]
    cols = (4 * np.arange(128)[:, None] - 1 + np.arange(6)[None, :])  # [128,6]
    valid = (cols >= 0) & (cols < N)
    t = S.T[np.clip(cols, 0, N - 1)]          # [128, 6, RS]
    t[~valid] = 0.0
    flat = np.zeros((128, F), np.float32)
    flat[:, 0:B] = t[:, 0]                      # GL
    flat[:, S0:S0 + 4 * B] = t[:, 1:5].reshape(128, 4 * B)
    flat[:, 5 * B + 2:6 * B + 2] = t[:, 5]      # GR
    return np.ascontiguousarray(flat, dtype=np.float32)


def _build(nc, tile, mybir, bass, scal):
    f32 = mybir.dt.float32
    u32 = mybir.dt.uint32
    OP = mybir.AluOpType
    ops = _register_ops()
    APHI, BSQ, CLIN, DLIN, SEL = (ops[k] for k in
                                  ("APHI", "BSQ", "CLIN", "DLIN", "SEL"))

    bf16 = mybir.dt.bfloat16
    BF = bf16 if os.environ.get("ADR_BF16", "1") == "1" else f32
    AF = mybir.ActivationFunctionType
    u0s_d = nc.dram_tensor("u0s", [128, F], f32, kind="ExternalInput").ap()
    ppc_d = nc.dram_tensor("ppc", [128, 16], f32, kind="ExternalInput").ap()
    wr_d = nc.dram_tensor("wr", [128, 128], f32, kind="ExternalInput").ap()
    wl_d = nc.dram_tensor("wl", [128, 128], f32, kind="ExternalInput").ap()
    nbrs_d = nc.dram_tensor("nbrs", [1, 2], u32, kind="ExternalInput").ap()
    rsel_d = nc.dram_tensor("rsel", [1, 2], u32, kind="ExternalInput").ap()
    msk_d = nc.dram_tensor("msk", [128, 2 * B], f32, kind="ExternalInput").ap()
    out_d = nc.dram_tensor("out", [NSTEPS, 128, 4, 64], f32,
                           kind="ExternalOutput").ap()

    ca, cb = scal["ca"], scal["cb"]

    # ghost sync every K steps (synchronous: state-t bands must merge into the
    # state-t tile before step t+1 — any lag breaks time-consistency)
    nsync = [t for t in range(K, NSTEPS, K)]

    with tile.TileContext(nc) as tc:
        with tc.tile_pool(name="state", bufs=1) as sp, \
             tc.tile_pool(name="tmp", bufs=2) as tp, \
             tc.tile_pool(name="psum", bufs=2, space="PSUM") as pp, \
             tc.tile_pool(name="dram", bufs=1, space="DRAM") as dp:

            tA = sp.tile([128, F], f32, tag="tA")
            tB = sp.tile([128, F], f32, tag="tB")
            ppc = sp.tile([128, 16], f32, tag="ppc")
            msk = sp.tile([128, 2 * B], f32, tag="msk")
            wr = sp.tile([128, 128], f32, tag="wr")
            wl = sp.tile([128, 128], f32, tag="wl")

            cc_in = dp.tile([2, 128, 4, K], f32, tag="ccin")
            cc_outs = {t: dp.tile([16 * 128, 4, K], f32, tag=f"ccout{t}",
                                  name=f"ccout{t}", addr_space="Shared")
                       for t in nsync}


            nc.sync.dma_start(tA[:], u0s_d[:])
            nc.sync.dma_start(ppc[:], ppc_d[:])
            nc.sync.dma_start(msk[:], msk_d[:])
            nc.sync.dma_start(wr[:], wr_d[:])
            nc.sync.dma_start(wl[:], wl_d[:])

            rp = nc.alloc_registers("rprev")
            nc.regs_load(rp, nbrs_d[0:1, 0:1])
            sv_prev = nc.snap(rp, min_val=0, max_val=15 * 128)
            rn = nc.alloc_registers("rnext")
            nc.regs_load(rn, nbrs_d[0:1, 1:2])
            sv_next = nc.snap(rn, min_val=0, max_val=15 * 128)
            # per-core Neumann source rows (core 0: 17 else 16; core 7: 78 else 79)
            rt0 = nc.alloc_registers("rtop")
            nc.regs_load(rt0, rsel_d[0:1, 0:1])
            sv_rtop = nc.snap(rt0, min_val=K, max_val=K + 1)
            rb0 = nc.alloc_registers("rbot")
            nc.regs_load(rb0, rsel_d[0:1, 1:2])
            sv_rbot = nc.snap(rb0, min_val=K + 62, max_val=K + 63)

            s_ = ppc[:, 3:4]; h_ = ppc[:, 4:5]
            ifA = ppc[:, 9:10]; ifB = ppc[:, 10:11]; ifC = ppc[:, 11:12]
            cD = ppc[:, 12:13]; cE = ppc[:, 13:14]
            one_ = ppc[:, 14:15]

            cur, nxt = tA, tB
            # ghost refresh for the initial state (u0 pack already provides
            # valid GL/GR, so nothing needed before step 1)
            for t in range(1, NSTEPS + 1):
                # views into the flat state
                Tc = cur[:, S0:S0 + 4 * B]
                Up = cur[:, S0 - 1:S0 + 4 * B - 1]
                Dn = cur[:, S0 + 1:S0 + 4 * B + 1]
                GLc = cur[:, 0:B]
                GRc = cur[:, 5 * B + 2:6 * B + 2]
                TnV = nxt[:, S0:S0 + 4 * B]

                PU = tp.tile([128, 4 * B], BF, tag="PU")
                PD = tp.tile([128, 4 * B], BF, tag="PD")
                PL = tp.tile([128, 4 * B], BF, tag="PL")
                PR = tp.tile([128, 4 * B], f32, tag="PR")
                S1 = tp.tile([128, 4 * B], BF, tag="S1")
                S2 = tp.tile([128, 4 * B], BF, tag="S2")
                I1 = tp.tile([128, B], f32, tag="I1")

                # interface precompute (reads OLD state only): one SEL
                # custom op at partition base 0 (custom scalar APs only work
                # at base 0): I1 = ifA*GRc + ifB*b3 (nonzero only at p63)
                nc.vector._custom_dve(
                    SEL, out=I1[:], in0=GRc,
                    in1=cur[:, S0 + 2 * B:S0 + 3 * B], s0=ifA, s1=ifB)

                # neighbor-grouped fused passes (DVE) + bf16 sum tree.
                # Ghost-column parts (CG/DG) are scheduled late so the ghost
                # refresh of the previous step can overlap the leading ops.
                nc.vector._custom_dve(APHI, out=PU[:], in0=Tc, in1=Up,
                                      s0=h_, s1=s_, imm2=2.0 * DX)
                nc.vector._custom_dve(BSQ, out=PD[:], in0=Tc, in1=Dn,
                                      s0=h_, s1=s_)
                # DLIN over blocks 1-3 (R = blocks 2-4), carries phi
                nc.vector._custom_dve(DLIN, out=PR[:, 0:3 * B],
                                      in0=cur[:, S0:S0 + 3 * B],
                                      in1=cur[:, S0 + B:S0 + 4 * B],
                                      s0=h_, s1=s_, imm2=-4.0)
                nc.vector.tensor_tensor(S1[:], PU[:], PD[:], OP.add)
                # CLIN over blocks 2-4 (L = blocks 1-3), carries phi part 3
                nc.vector._custom_dve(CLIN, out=PL[:, B:4 * B],
                                      in0=cur[:, S0 + B:S0 + 4 * B],
                                      in1=cur[:, S0:S0 + 3 * B],
                                      s0=h_, s1=s_, imm2=2.0 * DX)
                # ghost parts (same ops so phi lands exactly once per point):
                # block 1 L from GL, block 4 R from GR
                nc.vector._custom_dve(CLIN, out=PL[:, 0:B],
                                      in0=cur[:, S0:S0 + B], in1=GLc,
                                      s0=h_, s1=s_, imm2=2.0 * DX)
                nc.vector._custom_dve(DLIN, out=PR[:, 3 * B:4 * B],
                                      in0=cur[:, S0 + 3 * B:S0 + 4 * B],
                                      in1=GRc, s0=h_, s1=s_, imm2=-4.0)
                nc.vector.tensor_tensor(S2[:], S1[:], PL[:], OP.add)
                nc.vector.tensor_tensor(TnV, S2[:], PR[:], OP.add)

                # interface blend into b4: SEL in-place, full partitions
                # (ifC = 1 except p63=0; I1 nonzero only at p63)
                b4v = nxt[:, S0 + 3 * B:S0 + 4 * B]
                nc.vector._custom_dve(SEL, out=b4v, in0=b4v, in1=I1[:],
                                      s0=ifC, s1=one_)

                # row boundary (Neumann) with per-core dynamic source row
                # (middle cores self-copy).  Blocks 1-3 go on ACT right
                # after TnV; block 4 (which must wait for the interface
                # blend) goes on Vector so the ACT queue is never stalled.
                nx4 = nxt[:, S0:S0 + 4 * B].rearrange("p (b i) -> p b i", b=4)
                nc.scalar.copy(nx4[:, 0:3, K:K + 1],
                               nx4[:, 0:3, bass.ds(sv_rtop, 1)])
                nc.scalar.copy(nx4[:, 0:3, K + 63:K + 64],
                               nx4[:, 0:3, bass.ds(sv_rbot, 1)])

                # col 511 (p127) Neumann: SEL b4 <- cE*b4 + cD*b3 in-place,
                # full partitions.  Reads b3 rows post-row-copy; the block-4
                # row copies afterwards are idempotent at p127.
                nc.vector._custom_dve(
                    SEL, out=b4v, in0=b4v,
                    in1=nxt[:, S0 + 2 * B:S0 + 3 * B], s0=cE, s1=cD)
                nc.vector.tensor_scalar_add(
                    nx4[:, 3:4, K:K + 1], nx4[:, 3:4, bass.ds(sv_rtop, 1)],
                    0.0)
                nc.vector.tensor_scalar_add(
                    nx4[:, 3:4, K + 63:K + 64],
                    nx4[:, 3:4, bass.ds(sv_rbot, 1)], 0.0)

                # column boundary: col 0 (p0) on ACT
                nc.scalar.copy(nxt[0:1, S0:S0 + B], nxt[0:1, S0 + B:S0 + 2 * B])

                # ghost row sync (blocking; gpsimd queue keeps Sync free).
                # Bands carry only the 4 state blocks; GL/GR are rebuilt by
                # the ghost-column matmuls placed AFTER the merge below.
                if t in nsync:
                    cc_out = cc_outs[t]
                    nc.gpsimd.dma_start(cc_in[0], nx4[:, :, K:2 * K])
                    nc.gpsimd.dma_start(cc_in[1], nx4[:, :, 64:64 + K])
                    nc.gpsimd.collective_compute(
                        "AllGather", OP.bypass,
                        replica_groups=[list(range(NCORES))],
                        ins=[cc_in[:]], outs=[cc_out[:]])
                    nc.gpsimd.dma_start(nx4[:, :, 0:K],
                                        cc_out[bass.ds(sv_prev, 128)])
                    nc.gpsimd.dma_start(nx4[:, :, 64 + K:64 + 2 * K],
                                        cc_out[bass.ds(sv_next, 128)])

                # ghost column refresh via partition-shift matmuls (full
                # rows).  At sync steps this reads the merged tile, so the
                # new GL/GR include fresh ghost-row values.
                psR = pp.tile([128, B], f32, tag="psR")
                psL = pp.tile([128, B], f32, tag="psL")
                nc.tensor.matmul(psR[:], wr[:], nxt[:, S0:S0 + B],
                                 start=True, stop=True)
                nc.tensor.matmul(psL[:], wl[:], nxt[:, S0 + 3 * B:S0 + 4 * B],
                                 start=True, stop=True)
                nc.scalar.copy(nxt[:, 5 * B + 2:6 * B + 2], psR[:])
                nc.scalar.copy(nxt[:, 0:B], psL[:])

                # output: owned rows (the read has two steps of slack)
                nc.sync.dma_start(out_d[t - 1], nx4[:, :, K:K + 64])

                cur, nxt = nxt, cur
    return nc


def _ensure_ntff_hook():
    """Provide antenv.axon_hooks (missing in this image) so bass_utils can
    NTFF-profile under axon."""
    import sys
    import types
    try:
        from antenv.axon_hooks import get_axon_ntff_profile_hook  # noqa: F401
        return
    except ImportError:
        pass
    mod = types.ModuleType("antenv.axon_hooks")
    mod._hook = None

    def set_axon_ntff_profile_hook(h):
        mod._hook = h

    def get_axon_ntff_profile_hook():
        return mod._hook

    mod.set_axon_ntff_profile_hook = set_axon_ntff_profile_hook
    mod.get_axon_ntff_profile_hook = get_axon_ntff_profile_hook
    sys.modules["antenv.axon_hooks"] = mod
    import antenv
    antenv.axon_hooks = mod
    try:
        from trn_agent_boot.trn_boot import _ntff_profile_via_ctypes
        hook = _ntff_profile_via_ctypes("/opt/axon/libaxon_pjrt.so")
        if hook is not None:
            mod._hook = hook
    except Exception:
        pass


def kernel(u0, k1, k2, alpha1, alpha2):
    global LAST_EXEC_NS, LAST_RESULT
    import concourse.bacc as bacc
    import concourse.bass as bass
    import concourse.tile as tile
    import concourse.mybir as mybir
    from concourse.bass_utils import run_bass_kernel_spmd

    u0 = np.asarray(u0, dtype=np.float32)
    k1f = float(np.asarray(k1).reshape(-1)[0])
    k2f = float(np.asarray(k2).reshape(-1)[0])
    a1f = float(np.asarray(alpha1).reshape(-1)[0])
    a2f = float(np.asarray(alpha2).reshape(-1)[0])

    dx2 = DX * DX
    scal = {"ca": k1f / (k1f + k2f), "cb": k2f / (k1f + k2f)}

    nc = bacc.Bacc(
        "TRN2", target_bir_lowering=False, debug=False,
        num_devices=NCORES,
    )
    _build(nc, tile, mybir, bass, scal)
    nc.compile()

    left = np.arange(128) < 64
    s = np.where(left, DT * a1f / dx2, DT * a2f / dx2).astype(np.float32)
    h = np.where(left, DT * k1f / (2 * DX), DT * k2f / (2 * DX)).astype(np.float32)
    g = np.where(left, DT * k1f, DT * k2f).astype(np.float32)
    WR = np.eye(128, k=-1, dtype=np.float32)   # out[m] = in[m+1]
    WL = np.eye(128, k=+1, dtype=np.float32)   # out[m] = in[m-1]

    m63 = (np.arange(128) == 63).astype(np.float32)
    m127 = (np.arange(128) == 127).astype(np.float32)
    in_maps = []
    for c in range(NCORES):
        ppc = np.zeros((128, 16), np.float32)
        ppc[:, 0] = g
        ppc[:, 1] = -g
        ppc[:, 2] = 1.0 - 4.0 * s + g
        ppc[:, 3] = s
        ppc[:, 4] = h
        ppc[:, 9] = m63 * scal["ca"]       # ifA
        ppc[:, 10] = m63 * scal["cb"]      # ifB
        ppc[:, 11] = 1.0 - m63             # ifC
        ppc[:, 12] = m127                  # cD
        ppc[:, 13] = 1.0 - m127            # cE
        ppc[:, 14] = 1.0                   # ones (SEL s1 for interface)
        prev_off = (2 * (c - 1) + 1) * 128 if c > 0 else 0
        next_off = (2 * (c + 1)) * 128 if c < NCORES - 1 else 0
        rtop = K + 1 if c == 0 else K
        rbot = K + 62 if c == NCORES - 1 else K + 63
        in_maps.append({
            "u0s": _pack_core(u0, c),
            "ppc": ppc,
            "wr": WR,
            "wl": WL,
            "nbrs": np.array([[prev_off, next_off]], dtype=np.uint32),
            "rsel": np.array([[rtop, rbot]], dtype=np.uint32),
            "msk": np.concatenate([
                np.broadcast_to((m63 * scal["ca"])[:, None], (128, B)),
                np.broadcast_to((m63 * scal["cb"])[:, None], (128, B)),
            ], axis=1).astype(np.float32).copy(),
        })

    trace = os.environ.get("ADR_TRACE", "0") == "1"
    if trace:
        _ensure_ntff_hook()
    res = run_bass_kernel_spmd(
        nc, in_maps, core_ids=list(range(NCORES)), trace=trace)
    LAST_EXEC_NS = res.exec_time_ns
    LAST_RESULT = res

    full = np.zeros((NSTEPS, N, N), np.float32)
    for c in range(NCORES):
        arr = np.asarray(res.results[c]["out"]).reshape(NSTEPS, 128, 4, 64)
        full[:, 64 * c:64 * (c + 1), :] = (
            arr.transpose(0, 3, 1, 2).reshape(NSTEPS, 64, 512))
    return full


# revision 50
# speedup vs baseline: 1.1686x; 1.0070x over previous
"""Trainium2 Bass kernel for nn_AdvectionDiffusionReaction2M (v3).

Advection-diffusion-reaction on a 512x512 grid, 199 sequential steps, output =
all intermediate states (199,512,512) f32.

Sharding: rows split 8 ways (64 rows/core) with 16-row ghost zones refreshed
by an AllGather every 16 steps.  SBUF layout per core: flat [128, 6B+2] f32
per state buffer:
    [ GL (B) | pad | b1 b2 b3 b4 (4B) | pad | GR (B) ]
partition p = column group (cols 4p..4p+3 at blocks b1..b4), GL/GR = ghost
columns 4p-1 / 4p+4, i = stored row (96 = 16 ghost + 64 + 16 ghost).  The two
pad columns make the Up/Dn offset views disjoint from GL/GR, so the ghost
column refresh (PE partition-shift matmuls + PSUM->SBUF copies) overlaps the
next step's leading DVE ops instead of serializing the whole step.

The update is regrouped per neighbor with Tc-dependent coefficients
   Tn = Up*(s+h*Tc^2) + Dn*(s-h*Tc^2) + L*(s-h*Tc) + R*(s+h*Tc) + phi(Tc)
   phi = Tc + g*(Tc^3-Tc^2+Tc),  g = h*2dx
computed by fused custom DVE ops (block-edge rows are sacrificial ghost rows,
so row-crossing garbage in Up/Dn is harmless).  L and R are split into an
interior part (blocks) and a one-block ghost part (GL/GR) per pass.
"""

import os
import numpy as np

N = 512
DX = 1.0 / (N - 1)
DT = 1e-7
MB = 256
NCORES = 8
K = 16                      # ghost depth (rows)
RS = 64 + 2 * K             # stored rows per core (96)
NSTEPS = int(os.environ.get("ADR_NSTEPS", "199"))
B = RS                      # block stride in flat free dim
F = 6 * B + 2               # flat state width: GL|pad|b1..b4|pad|GR
S0 = B + 1                  # flat offset of block 1 (state region start)

LAST_EXEC_NS = None
LAST_RESULT = None

_OPS_REGISTERED = {}


def _register_ops():
    """Register custom DVE ops (runtime registration into dve_ops.OPS)."""
    if _OPS_REGISTERED:
        return _OPS_REGISTERED
    import concourse.dve_ops as dve_ops
    from concourse.dve_ops import DveOp, OPS
    from concourse.dve_spec import Spec, Src0, Src1, C0, C1, C2, One, sq, lower
    from concourse.dve_uop import DveOpSpec

    def make_op(name, body, reference):
        for op in OPS:
            if op.name == name:
                return op
        spec = Spec(body=body, reference=reference)
        shas = {}
        for ver in ("v3", "v4"):
            uops = lower(spec, ver=ver)
            tmp = DveOpSpec(name=name, opcode=0, uops=uops, rd1_en=True)
            shas[ver] = tmp.sha(ver)
        op = DveOp(name, spec, subdim=False, uops_sha=shas)
        OPS.append(op)
        dve_ops._SUB_OPCODE_FOR_NAME[name] = (
            dve_ops._CUSTOM_DVE_ROW_BASE + len(OPS) - 1)
        assert dve_ops._SUB_OPCODE_FOR_NAME[name] < 0x20, "opcode row overflow"
        dve_ops.CUSTOM_DVE_SPECS[name] = spec
        return op

    q = sq(Src0)
    gc = C0 * C2                          # g = h * 2dx (hoisted mult)
    # out = Up*(s + h*Tc^2) + g*(Tc^2 - Tc)*Tc      [phi part 1: g(Tc^3-Tc^2)]
    _OPS_REGISTERED["APHI"] = make_op(
        "ADR_APHI",
        Src1 * (C1 + q * C0) + (q - Src0) * gc * Src0,
        lambda in0, in1, s0, s1, imm2:
            in1 * (s1 + in0**2 * s0)
            + (in0**2 - in0) * (s0 * imm2) * in0)
    # out = Dn*(s - h*Tc^2)
    _OPS_REGISTERED["BSQ"] = make_op(
        "ADR_BSQ", Src1 * (C1 - q * C0),
        lambda in0, in1, s0, s1: in1 * (s1 - in0**2 * s0))
    # out = L*(s - h*Tc) + (h*Tc)*2dx               [phi part 3: g*Tc]
    _a = Src0 * C0
    _OPS_REGISTERED["CLIN"] = make_op(
        "ADR_CLIN", Src1 * (C1 - _a) + _a * C2,
        lambda in0, in1, s0, s1, imm2:
            in1 * (s1 - in0 * s0) + in0 * s0 * imm2)
    # out = R*(s + h*Tc) + Tc + (-4)*s*Tc           [phi part 2: (1-4s)Tc]
    _OPS_REGISTERED["DLIN"] = make_op(
        "ADR_DLIN", Src1 * (C1 + _a) + Src0 + Src0 * C1 * C2,
        lambda in0, in1, s0, s1, imm2:
            in1 * (s1 + in0 * s0) + in0 + in0 * s1 * imm2)
    # out = Src0*C0 + Src1*C1  (masked blend / select)
    _OPS_REGISTERED["SEL"] = make_op(
        "ADR_SEL", Src0 * C0 + Src1 * C1,
        lambda in0, in1, s0, s1: in0 * s0 + in1 * s1)
    return _OPS_REGISTERED


def _pack_core(G, c):
    """Full grid (512,512) -> per-core flat tile [128, F] (f32, zero padded).

    Layout per partition p: [GL | 0 | b1 b2 b3 b4 | 0 | GR] where block bj
    holds column 4p+j-1 over the RS stored rows and GL/GR hold cols 4p-1 /
    4p+4.
    """
    lo = 64 * c - K
    S = np.zeros((RS, N), np.float32)
    g0, g1 = max(lo, 0), min(lo + RS, N)
    S[g0 - lo: g1 - lo] = G[g0:g1]
    cols = (4 * np.arange(128)[:, None] - 1 + np.arange(6)[None, :])  # [128,6]
    valid = (cols >= 0) & (cols < N)
    t = S.T[np.clip(cols, 0, N - 1)]          # [128, 6, RS]
    t[~valid] = 0.0
    flat = np.zeros((128, F), np.float32)
    flat[:, 0:B] = t[:, 0]                      # GL
    flat[:, S0:S0 + 4 * B] = t[:, 1:5].reshape(128, 4 * B)
    flat[:, 5 * B + 2:6 * B + 2] = t[:, 5]      # GR
    return np.ascontiguousarray(flat, dtype=np.float32)


def _build(nc, tile, mybir, bass, scal):
    f32 = mybir.dt.float32
    u32 = mybir.dt.uint32
    OP = mybir.AluOpType
    ops = _register_ops()
    APHI, BSQ, CLIN, DLIN, SEL = (ops[k] for k in
                                  ("APHI", "BSQ", "CLIN", "DLIN", "SEL"))

    bf16 = mybir.dt.bfloat16
    BF = bf16 if os.environ.get("ADR_BF16", "1") == "1" else f32
    AF = mybir.ActivationFunctionType
    u0s_d = nc.dram_tensor("u0s", [128, F], f32, kind="ExternalInput").ap()
    ppc_d = nc.dram_tensor("ppc", [128, 16], f32, kind="ExternalInput").ap()
    wr_d = nc.dram_tensor("wr", [128, 128], f32, kind="ExternalInput").ap()
    wl_d = nc.dram_tensor("wl", [128, 128], f32, kind="ExternalInput").ap()
    nbrs_d = nc.dram_tensor("nbrs", [1, 2], u32, kind="ExternalInput").ap()
    rsel_d = nc.dram_tensor("rsel", [1, 2], u32, kind="ExternalInput").ap()
    msk_d = nc.dram_tensor("msk", [128, 2 * B], f32, kind="ExternalInput").ap()
    out_d = nc.dram_tensor("out", [NSTEPS, 128, 4, 64], f32,
                           kind="ExternalOutput").ap()

    ca, cb = scal["ca"], scal["cb"]

    # ghost sync every K steps (synchronous: state-t bands must merge into the
    # state-t tile before step t+1 — any lag breaks time-consistency)
    nsync = [t for t in range(K, NSTEPS, K)]

    with tile.TileContext(nc) as tc:
        with tc.tile_pool(name="state", bufs=1) as sp, \
             tc.tile_pool(name="tmp", bufs=2) as tp, \
             tc.tile_pool(name="psum", bufs=2, space="PSUM") as pp, \
             tc.tile_pool(name="dram", bufs=1, space="DRAM") as dp:

            tA = sp.tile([128, F], f32, tag="tA")
            tB = sp.tile([128, F], f32, tag="tB")
            ppc = sp.tile([128, 16], f32, tag="ppc")
            msk = sp.tile([128, 2 * B], f32, tag="msk")
            wr = sp.tile([128, 128], f32, tag="wr")
            wl = sp.tile([128, 128], f32, tag="wl")

            cc_in = dp.tile([2, 128, 4, K], f32, tag="ccin")
            cc_outs = {t: dp.tile([16 * 128, 4, K], f32, tag=f"ccout{t}",
                                  name=f"ccout{t}", addr_space="Shared")
                       for t in nsync}


            nc.sync.dma_start(tA[:], u0s_d[:])
            nc.sync.dma_start(ppc[:], ppc_d[:])
            nc.sync.dma_start(msk[:], msk_d[:])
            nc.sync.dma_start(wr[:], wr_d[:])
            nc.sync.dma_start(wl[:], wl_d[:])

            rp = nc.alloc_registers("rprev")
            nc.regs_load(rp, nbrs_d[0:1, 0:1])
            sv_prev = nc.snap(rp, min_val=0, max_val=15 * 128)
            rn = nc.alloc_registers("rnext")
            nc.regs_load(rn, nbrs_d[0:1, 1:2])
            sv_next = nc.snap(rn, min_val=0, max_val=15 * 128)
            # per-core Neumann source rows (core 0: 17 else 16; core 7: 78 else 79)
            rt0 = nc.alloc_registers("rtop")
            nc.regs_load(rt0, rsel_d[0:1, 0:1])
            sv_rtop = nc.snap(rt0, min_val=K, max_val=K + 1)
            rb0 = nc.alloc_registers("rbot")
            nc.regs_load(rb0, rsel_d[0:1, 1:2])
            sv_rbot = nc.snap(rb0, min_val=K + 62, max_val=K + 63)

            s_ = ppc[:, 3:4]; h_ = ppc[:, 4:5]
            ifA = ppc[:, 9:10]; ifB = ppc[:, 10:11]; ifC = ppc[:, 11:12]
            cD = ppc[:, 12:13]; cE = ppc[:, 13:14]
            one_ = ppc[:, 14:15]

            # warm-up AllGather (tiny payload): pays the CC cold-start cost
            # while the first steps compute, so the step-16 collective runs
            # closer to warm latency.
            warm_in = dp.tile([1, 64], f32, tag="warmin")
            warm_out = dp.tile([8, 64], f32, tag="warmout", name="warmout",
                               addr_space="Shared")
            nc.gpsimd.collective_compute(
                "AllGather", OP.bypass,
                replica_groups=[list(range(NCORES))],
                ins=[warm_in[:]], outs=[warm_out[:]])

            cur, nxt = tA, tB
            # ghost refresh for the initial state (u0 pack already provides
            # valid GL/GR, so nothing needed before step 1)
            for t in range(1, NSTEPS + 1):
                # views into the flat state
                Tc = cur[:, S0:S0 + 4 * B]
                Up = cur[:, S0 - 1:S0 + 4 * B - 1]
                Dn = cur[:, S0 + 1:S0 + 4 * B + 1]
                GLc = cur[:, 0:B]
                GRc = cur[:, 5 * B + 2:6 * B + 2]
                TnV = nxt[:, S0:S0 + 4 * B]

                PU = tp.tile([128, 4 * B], BF, tag="PU")
                PD = tp.tile([128, 4 * B], BF, tag="PD")
                PL = tp.tile([128, 4 * B], BF, tag="PL")
                PR = tp.tile([128, 4 * B], f32, tag="PR")
                S1 = tp.tile([128, 4 * B], BF, tag="S1")
                S2 = tp.tile([128, 4 * B], BF, tag="S2")
                I1 = tp.tile([128, B], f32, tag="I1")

                # interface precompute (reads OLD state only): one SEL
                # custom op at partition base 0 (custom scalar APs only work
                # at base 0): I1 = ifA*GRc + ifB*b3 (nonzero only at p63)
                nc.vector._custom_dve(
                    SEL, out=I1[:], in0=GRc,
                    in1=cur[:, S0 + 2 * B:S0 + 3 * B], s0=ifA, s1=ifB)

                # neighbor-grouped fused passes (DVE) + bf16 sum tree.
                # Ghost-column parts (CG/DG) are scheduled late so the ghost
                # refresh of the previous step can overlap the leading ops.
                nc.vector._custom_dve(APHI, out=PU[:], in0=Tc, in1=Up,
                                      s0=h_, s1=s_, imm2=2.0 * DX)
                nc.vector._custom_dve(BSQ, out=PD[:], in0=Tc, in1=Dn,
                                      s0=h_, s1=s_)
                # DLIN over blocks 1-3 (R = blocks 2-4), carries phi
                nc.vector._custom_dve(DLIN, out=PR[:, 0:3 * B],
                                      in0=cur[:, S0:S0 + 3 * B],
                                      in1=cur[:, S0 + B:S0 + 4 * B],
                                      s0=h_, s1=s_, imm2=-4.0)
                nc.vector.tensor_tensor(S1[:], PU[:], PD[:], OP.add)
                # CLIN over blocks 2-4 (L = blocks 1-3), carries phi part 3
                nc.vector._custom_dve(CLIN, out=PL[:, B:4 * B],
                                      in0=cur[:, S0 + B:S0 + 4 * B],
                                      in1=cur[:, S0:S0 + 3 * B],
                                      s0=h_, s1=s_, imm2=2.0 * DX)
                # ghost parts (same ops so phi lands exactly once per point):
                # block 1 L from GL, block 4 R from GR
                nc.vector._custom_dve(CLIN, out=PL[:, 0:B],
                                      in0=cur[:, S0:S0 + B], in1=GLc,
                                      s0=h_, s1=s_, imm2=2.0 * DX)
                nc.vector._custom_dve(DLIN, out=PR[:, 3 * B:4 * B],
                                      in0=cur[:, S0 + 3 * B:S0 + 4 * B],
                                      in1=GRc, s0=h_, s1=s_, imm2=-4.0)
                nc.vector.tensor_tensor(S2[:], S1[:], PL[:], OP.add)
                nc.vector.tensor_tensor(TnV, S2[:], PR[:], OP.add)

                # interface blend into b4: SEL in-place, full partitions
                # (ifC = 1 except p63=0; I1 nonzero only at p63)
                b4v = nxt[:, S0 + 3 * B:S0 + 4 * B]
                nc.vector._custom_dve(SEL, out=b4v, in0=b4v, in1=I1[:],
                                      s0=ifC, s1=one_)

                # row boundary (Neumann) with per-core dynamic source row
                # (middle cores self-copy).  Blocks 1-3 go on ACT right
                # after TnV; block 4 (which must wait for the interface
                # blend) goes on Vector so the ACT queue is never stalled.
                nx4 = nxt[:, S0:S0 + 4 * B].rearrange("p (b i) -> p b i", b=4)
                nc.scalar.copy(nx4[:, 0:3, K:K + 1],
                               nx4[:, 0:3, bass.ds(sv_rtop, 1)])
                nc.scalar.copy(nx4[:, 0:3, K + 63:K + 64],
                               nx4[:, 0:3, bass.ds(sv_rbot, 1)])

                # col 511 (p127) Neumann: SEL b4 <- cE*b4 + cD*b3 in-place,
                # full partitions.  Reads b3 rows post-row-copy; the block-4
                # row copies afterwards are idempotent at p127.
                nc.vector._custom_dve(
                    SEL, out=b4v, in0=b4v,
                    in1=nxt[:, S0 + 2 * B:S0 + 3 * B], s0=cE, s1=cD)
                nc.vector.tensor_scalar_add(
                    nx4[:, 3:4, K:K + 1], nx4[:, 3:4, bass.ds(sv_rtop, 1)],
                    0.0)
                nc.vector.tensor_scalar_add(
                    nx4[:, 3:4, K + 63:K + 64],
                    nx4[:, 3:4, bass.ds(sv_rbot, 1)], 0.0)

                # column boundary: col 0 (p0) on ACT
                nc.scalar.copy(nxt[0:1, S0:S0 + B], nxt[0:1, S0 + B:S0 + 2 * B])

                # ghost row sync (blocking; gpsimd queue keeps Sync free).
                # Bands carry only the 4 state blocks; GL/GR are rebuilt by
                # the ghost-column matmuls placed AFTER the merge below.
                if t in nsync:
                    cc_out = cc_outs[t]
                    nc.gpsimd.dma_start(cc_in[0], nx4[:, :, K:2 * K])
                    nc.gpsimd.dma_start(cc_in[1], nx4[:, :, 64:64 + K])
                    nc.gpsimd.collective_compute(
                        "AllGather", OP.bypass,
                        replica_groups=[list(range(NCORES))],
                        ins=[cc_in[:]], outs=[cc_out[:]])
                    nc.gpsimd.dma_start(nx4[:, :, 0:K],
                                        cc_out[bass.ds(sv_prev, 128)])
                    nc.gpsimd.dma_start(nx4[:, :, 64 + K:64 + 2 * K],
                                        cc_out[bass.ds(sv_next, 128)])

                # ghost column refresh via partition-shift matmuls (full
                # rows).  At sync steps this reads the merged tile, so the
                # new GL/GR include fresh ghost-row values.
                psR = pp.tile([128, B], f32, tag="psR")
                psL = pp.tile([128, B], f32, tag="psL")
                nc.tensor.matmul(psR[:], wr[:], nxt[:, S0:S0 + B],
                                 start=True, stop=True)
                nc.tensor.matmul(psL[:], wl[:], nxt[:, S0 + 3 * B:S0 + 4 * B],
                                 start=True, stop=True)
                nc.scalar.copy(nxt[:, 5 * B + 2:6 * B + 2], psR[:])
                nc.scalar.copy(nxt[:, 0:B], psL[:])

                # output: owned rows (the read has two steps of slack)
                nc.sync.dma_start(out_d[t - 1], nx4[:, :, K:K + 64])

                cur, nxt = nxt, cur
    return nc


def _ensure_ntff_hook():
    """Provide antenv.axon_hooks (missing in this image) so bass_utils can
    NTFF-profile under axon."""
    import sys
    import types
    try:
        from antenv.axon_hooks import get_axon_ntff_profile_hook  # noqa: F401
        return
    except ImportError:
        pass
    mod = types.ModuleType("antenv.axon_hooks")
    mod._hook = None

    def set_axon_ntff_profile_hook(h):
        mod._hook = h

    def get_axon_ntff_profile_hook():
        return mod._hook

    mod.set_axon_ntff_profile_hook = set_axon_ntff_profile_hook
    mod.get_axon_ntff_profile_hook = get_axon_ntff_profile_hook
    sys.modules["antenv.axon_hooks"] = mod
    import antenv
    antenv.axon_hooks = mod
    try:
        from trn_agent_boot.trn_boot import _ntff_profile_via_ctypes
        hook = _ntff_profile_via_ctypes("/opt/axon/libaxon_pjrt.so")
        if hook is not None:
            mod._hook = hook
    except Exception:
        pass


def kernel(u0, k1, k2, alpha1, alpha2):
    global LAST_EXEC_NS, LAST_RESULT
    import concourse.bacc as bacc
    import concourse.bass as bass
    import concourse.tile as tile
    import concourse.mybir as mybir
    from concourse.bass_utils import run_bass_kernel_spmd

    u0 = np.asarray(u0, dtype=np.float32)
    k1f = float(np.asarray(k1).reshape(-1)[0])
    k2f = float(np.asarray(k2).reshape(-1)[0])
    a1f = float(np.asarray(alpha1).reshape(-1)[0])
    a2f = float(np.asarray(alpha2).reshape(-1)[0])

    dx2 = DX * DX
    scal = {"ca": k1f / (k1f + k2f), "cb": k2f / (k1f + k2f)}

    nc = bacc.Bacc(
        "TRN2", target_bir_lowering=False, debug=False,
        num_devices=NCORES,
    )
    _build(nc, tile, mybir, bass, scal)
    nc.compile()

    left = np.arange(128) < 64
    s = np.where(left, DT * a1f / dx2, DT * a2f / dx2).astype(np.float32)
    h = np.where(left, DT * k1f / (2 * DX), DT * k2f / (2 * DX)).astype(np.float32)
    g = np.where(left, DT * k1f, DT * k2f).astype(np.float32)
    WR = np.eye(128, k=-1, dtype=np.float32)   # out[m] = in[m+1]
    WL = np.eye(128, k=+1, dtype=np.float32)   # out[m] = in[m-1]

    m63 = (np.arange(128) == 63).astype(np.float32)
    m127 = (np.arange(128) == 127).astype(np.float32)
    in_maps = []
    for c in range(NCORES):
        ppc = np.zeros((128, 16), np.float32)
        ppc[:, 0] = g
        ppc[:, 1] = -g
        ppc[:, 2] = 1.0 - 4.0 * s + g
        ppc[:, 3] = s
        ppc[:, 4] = h
        ppc[:, 9] = m63 * scal["ca"]       # ifA
        ppc[:, 10] = m63 * scal["cb"]      # ifB
        ppc[:, 11] = 1.0 - m63             # ifC
        ppc[:, 12] = m127                  # cD
        ppc[:, 13] = 1.0 - m127            # cE
        ppc[:, 14] = 1.0                   # ones (SEL s1 for interface)
        prev_off = (2 * (c - 1) + 1) * 128 if c > 0 else 0
        next_off = (2 * (c + 1)) * 128 if c < NCORES - 1 else 0
        rtop = K + 1 if c == 0 else K
        rbot = K + 62 if c == NCORES - 1 else K + 63
        in_maps.append({
            "u0s": _pack_core(u0, c),
            "ppc": ppc,
            "wr": WR,
            "wl": WL,
            "nbrs": np.array([[prev_off, next_off]], dtype=np.uint32),
            "rsel": np.array([[rtop, rbot]], dtype=np.uint32),
            "msk": np.concatenate([
                np.broadcast_to((m63 * scal["ca"])[:, None], (128, B)),
                np.broadcast_to((m63 * scal["cb"])[:, None], (128, B)),
            ], axis=1).astype(np.float32).copy(),
        })

    trace = os.environ.get("ADR_TRACE", "0") == "1"
    if trace:
        _ensure_ntff_hook()
    res = run_bass_kernel_spmd(
        nc, in_maps, core_ids=list(range(NCORES)), trace=trace)
    LAST_EXEC_NS = res.exec_time_ns
    LAST_RESULT = res

    full = np.zeros((NSTEPS, N, N), np.float32)
    for c in range(NCORES):
        arr = np.asarray(res.results[c]["out"]).reshape(NSTEPS, 128, 4, 64)
        full[:, 64 * c:64 * (c + 1), :] = (
            arr.transpose(0, 3, 1, 2).reshape(NSTEPS, 64, 512))
    return full


# revision 51
# speedup vs baseline: 1.1958x; 1.0232x over previous
"""Trainium2 Bass kernel for nn_AdvectionDiffusionReaction2M (v3).

Advection-diffusion-reaction on a 512x512 grid, 199 sequential steps, output =
all intermediate states (199,512,512) f32.

Sharding: rows split 8 ways (64 rows/core) with 16-row ghost zones refreshed
by an AllGather every 16 steps.  SBUF layout per core: flat [128, 6B+2] f32
per state buffer:
    [ GL (B) | pad | b1 b2 b3 b4 (4B) | pad | GR (B) ]
partition p = column group (cols 4p..4p+3 at blocks b1..b4), GL/GR = ghost
columns 4p-1 / 4p+4, i = stored row (96 = 16 ghost + 64 + 16 ghost).  The two
pad columns make the Up/Dn offset views disjoint from GL/GR, so the ghost
column refresh (PE partition-shift matmuls + PSUM->SBUF copies) overlaps the
next step's leading DVE ops instead of serializing the whole step.

The update is regrouped per neighbor with Tc-dependent coefficients
   Tn = Up*(s+h*Tc^2) + Dn*(s-h*Tc^2) + L*(s-h*Tc) + R*(s+h*Tc) + phi(Tc)
   phi = Tc + g*(Tc^3-Tc^2+Tc),  g = h*2dx
computed by fused custom DVE ops (block-edge rows are sacrificial ghost rows,
so row-crossing garbage in Up/Dn is harmless).  L and R are split into an
interior part (blocks) and a one-block ghost part (GL/GR) per pass.
"""

import os
import numpy as np

N = 512
DX = 1.0 / (N - 1)
DT = 1e-7
MB = 256
NCORES = 8
K = 16                      # ghost depth (rows)
RS = 64 + 2 * K             # stored rows per core (96)
NSTEPS = int(os.environ.get("ADR_NSTEPS", "199"))
B = RS                      # block stride in flat free dim
F = 6 * B + 2               # flat state width: GL|pad|b1..b4|pad|GR
S0 = B + 1                  # flat offset of block 1 (state region start)

LAST_EXEC_NS = None
LAST_RESULT = None

_OPS_REGISTERED = {}


def _register_ops():
    """Register custom DVE ops (runtime registration into dve_ops.OPS)."""
    if _OPS_REGISTERED:
        return _OPS_REGISTERED
    import concourse.dve_ops as dve_ops
    from concourse.dve_ops import DveOp, OPS
    from concourse.dve_spec import Spec, Src0, Src1, C0, C1, C2, One, sq, lower
    from concourse.dve_uop import DveOpSpec

    def make_op(name, body, reference):
        for op in OPS:
            if op.name == name:
                return op
        spec = Spec(body=body, reference=reference)
        shas = {}
        for ver in ("v3", "v4"):
            uops = lower(spec, ver=ver)
            tmp = DveOpSpec(name=name, opcode=0, uops=uops, rd1_en=True)
            shas[ver] = tmp.sha(ver)
        op = DveOp(name, spec, subdim=False, uops_sha=shas)
        OPS.append(op)
        dve_ops._SUB_OPCODE_FOR_NAME[name] = (
            dve_ops._CUSTOM_DVE_ROW_BASE + len(OPS) - 1)
        assert dve_ops._SUB_OPCODE_FOR_NAME[name] < 0x20, "opcode row overflow"
        dve_ops.CUSTOM_DVE_SPECS[name] = spec
        return op

    q = sq(Src0)
    gc = C0 * C2                          # g = h * 2dx (hoisted mult)
    # out = Up*(s + h*Tc^2) + g*(Tc^2 - Tc)*Tc      [phi part 1: g(Tc^3-Tc^2)]
    _OPS_REGISTERED["APHI"] = make_op(
        "ADR_APHI",
        Src1 * (C1 + q * C0) + (q - Src0) * gc * Src0,
        lambda in0, in1, s0, s1, imm2:
            in1 * (s1 + in0**2 * s0)
            + (in0**2 - in0) * (s0 * imm2) * in0)
    # out = Dn*(s - h*Tc^2)
    _OPS_REGISTERED["BSQ"] = make_op(
        "ADR_BSQ", Src1 * (C1 - q * C0),
        lambda in0, in1, s0, s1: in1 * (s1 - in0**2 * s0))
    # out = L*(s - h*Tc) + (h*Tc)*2dx               [phi part 3: g*Tc]
    _a = Src0 * C0
    _OPS_REGISTERED["CLIN"] = make_op(
        "ADR_CLIN", Src1 * (C1 - _a) + _a * C2,
        lambda in0, in1, s0, s1, imm2:
            in1 * (s1 - in0 * s0) + in0 * s0 * imm2)
    # out = R*(s + h*Tc) + Tc + (-4)*s*Tc           [phi part 2: (1-4s)Tc]
    _OPS_REGISTERED["DLIN"] = make_op(
        "ADR_DLIN", Src1 * (C1 + _a) + Src0 + Src0 * C1 * C2,
        lambda in0, in1, s0, s1, imm2:
            in1 * (s1 + in0 * s0) + in0 + in0 * s1 * imm2)
    # out = Src0*C0 + Src1*C1  (masked blend / select)
    _OPS_REGISTERED["SEL"] = make_op(
        "ADR_SEL", Src0 * C0 + Src1 * C1,
        lambda in0, in1, s0, s1: in0 * s0 + in1 * s1)
    return _OPS_REGISTERED


def _pack_core(G, c):
    """Full grid (512,512) -> per-core flat tile [128, F] (f32, zero padded).

    Layout per partition p: [GL | 0 | b1 b2 b3 b4 | 0 | GR] where block bj
    holds column 4p+j-1 over the RS stored rows and GL/GR hold cols 4p-1 /
    4p+4.
    """
    lo = 64 * c - K
    S = np.zeros((RS, N), np.float32)
    g0, g1 = max(lo, 0), min(lo + RS, N)
    S[g0 - lo: g1 - lo] = G[g0:g1]
    cols = (4 * np.arange(128)[:, None] - 1 + np.arange(6)[None, :])  # [128,6]
    valid = (cols >= 0) & (cols < N)
    t = S.T[np.clip(cols, 0, N - 1)]          # [128, 6, RS]
    t[~valid] = 0.0
    flat = np.zeros((128, F), np.float32)
    flat[:, 0:B] = t[:, 0]                      # GL
    flat[:, S0:S0 + 4 * B] = t[:, 1:5].reshape(128, 4 * B)
    flat[:, 5 * B + 2:6 * B + 2] = t[:, 5]      # GR
    return np.ascontiguousarray(flat, dtype=np.float32)


def _build(nc, tile, mybir, bass, scal):
    f32 = mybir.dt.float32
    u32 = mybir.dt.uint32
    OP = mybir.AluOpType
    ops = _register_ops()
    APHI, BSQ, CLIN, DLIN, SEL = (ops[k] for k in
                                  ("APHI", "BSQ", "CLIN", "DLIN", "SEL"))

    bf16 = mybir.dt.bfloat16
    BF = bf16 if os.environ.get("ADR_BF16", "1") == "1" else f32
    AF = mybir.ActivationFunctionType
    u0s_d = nc.dram_tensor("u0s", [128, F], f32, kind="ExternalInput").ap()
    ppc_d = nc.dram_tensor("ppc", [128, 16], f32, kind="ExternalInput").ap()
    wr_d = nc.dram_tensor("wr", [128, 128], f32, kind="ExternalInput").ap()
    wl_d = nc.dram_tensor("wl", [128, 128], f32, kind="ExternalInput").ap()
    nbrs_d = nc.dram_tensor("nbrs", [1, 2], u32, kind="ExternalInput").ap()
    rsel_d = nc.dram_tensor("rsel", [1, 2], u32, kind="ExternalInput").ap()
    msk_d = nc.dram_tensor("msk", [128, 2 * B], f32, kind="ExternalInput").ap()
    out_d = nc.dram_tensor("out", [NSTEPS, 128, 4, 64], f32,
                           kind="ExternalOutput").ap()

    ca, cb = scal["ca"], scal["cb"]

    # ghost sync every K steps (synchronous: state-t bands must merge into the
    # state-t tile before step t+1 — any lag breaks time-consistency)
    nsync = [t for t in range(K, NSTEPS, K)]

    with tile.TileContext(nc) as tc:
        with tc.tile_pool(name="state", bufs=1) as sp, \
             tc.tile_pool(name="tmp", bufs=2) as tp, \
             tc.tile_pool(name="psum", bufs=2, space="PSUM") as pp, \
             tc.tile_pool(name="dram", bufs=1, space="DRAM") as dp:

            tA = sp.tile([128, F], f32, tag="tA")
            tB = sp.tile([128, F], f32, tag="tB")
            ppc = sp.tile([128, 16], f32, tag="ppc")
            msk = sp.tile([128, 2 * B], f32, tag="msk")
            wr = sp.tile([128, 128], f32, tag="wr")
            wl = sp.tile([128, 128], f32, tag="wl")

            cc_in = dp.tile([2, 128, 4, K], f32, tag="ccin")
            cc_outs = {t: dp.tile([16 * 128, 4, K], f32, tag=f"ccout{t}",
                                  name=f"ccout{t}", addr_space="Shared")
                       for t in nsync}


            nc.sync.dma_start(tA[:], u0s_d[:])
            nc.sync.dma_start(ppc[:], ppc_d[:])
            nc.sync.dma_start(msk[:], msk_d[:])
            nc.sync.dma_start(wr[:], wr_d[:])
            nc.sync.dma_start(wl[:], wl_d[:])

            rp = nc.alloc_registers("rprev")
            nc.regs_load(rp, nbrs_d[0:1, 0:1])
            sv_prev = nc.snap(rp, min_val=0, max_val=15 * 128)
            rn = nc.alloc_registers("rnext")
            nc.regs_load(rn, nbrs_d[0:1, 1:2])
            sv_next = nc.snap(rn, min_val=0, max_val=15 * 128)
            # per-core Neumann source rows (core 0: 17 else 16; core 7: 78 else 79)
            rt0 = nc.alloc_registers("rtop")
            nc.regs_load(rt0, rsel_d[0:1, 0:1])
            sv_rtop = nc.snap(rt0, min_val=K, max_val=K + 1)
            rb0 = nc.alloc_registers("rbot")
            nc.regs_load(rb0, rsel_d[0:1, 1:2])
            sv_rbot = nc.snap(rb0, min_val=K + 62, max_val=K + 63)

            s_ = ppc[:, 3:4]; h_ = ppc[:, 4:5]
            ifA = ppc[:, 9:10]; ifB = ppc[:, 10:11]; ifC = ppc[:, 11:12]
            cD = ppc[:, 12:13]; cE = ppc[:, 13:14]
            one_ = ppc[:, 14:15]

            # warm-up AllGather (tiny payload): pays the CC cold-start cost
            # while the first steps compute, so the step-16 collective runs
            # closer to warm latency.
            warm_in = dp.tile([1, 64], f32, tag="warmin")
            warm_out = dp.tile([8, 64], f32, tag="warmout", name="warmout",
                               addr_space="Shared")
            nc.gpsimd.collective_compute(
                "AllGather", OP.bypass,
                replica_groups=[list(range(NCORES))],
                ins=[warm_in[:]], outs=[warm_out[:]])

            cur, nxt = tA, tB
            # ghost refresh for the initial state (u0 pack already provides
            # valid GL/GR, so nothing needed before step 1)
            for t in range(1, NSTEPS + 1):
                # views into the flat state
                Tc = cur[:, S0:S0 + 4 * B]
                Up = cur[:, S0 - 1:S0 + 4 * B - 1]
                Dn = cur[:, S0 + 1:S0 + 4 * B + 1]
                GLc = cur[:, 0:B]
                GRc = cur[:, 5 * B + 2:6 * B + 2]
                TnV = nxt[:, S0:S0 + 4 * B]

                PU = tp.tile([128, 4 * B], BF, tag="PU")
                PD = tp.tile([128, 4 * B], BF, tag="PD")
                PL = tp.tile([128, 4 * B], BF, tag="PL")
                PR = tp.tile([128, 4 * B], f32, tag="PR")
                S1 = tp.tile([128, 4 * B], BF, tag="S1")
                S2 = tp.tile([128, 4 * B], BF, tag="S2")
                I1 = tp.tile([128, B], f32, tag="I1")

                # interface precompute (reads OLD state only): one SEL
                # custom op at partition base 0 (custom scalar APs only work
                # at base 0): I1 = ifA*GRc + ifB*b3 (nonzero only at p63)
                nc.vector._custom_dve(
                    SEL, out=I1[:], in0=GRc,
                    in1=cur[:, S0 + 2 * B:S0 + 3 * B], s0=ifA, s1=ifB)

                # neighbor-grouped fused passes (DVE) + bf16 sum tree.
                # Ghost-column parts (CG/DG) are scheduled late so the ghost
                # refresh of the previous step can overlap the leading ops.
                nc.vector._custom_dve(APHI, out=PU[:], in0=Tc, in1=Up,
                                      s0=h_, s1=s_, imm2=2.0 * DX)
                nc.vector._custom_dve(BSQ, out=PD[:], in0=Tc, in1=Dn,
                                      s0=h_, s1=s_)
                # DLIN over blocks 1-3 (R = blocks 2-4), carries phi
                nc.vector._custom_dve(DLIN, out=PR[:, 0:3 * B],
                                      in0=cur[:, S0:S0 + 3 * B],
                                      in1=cur[:, S0 + B:S0 + 4 * B],
                                      s0=h_, s1=s_, imm2=-4.0)
                nc.vector.tensor_tensor(S1[:], PU[:], PD[:], OP.add)
                # CLIN over blocks 2-4 (L = blocks 1-3), carries phi part 3
                nc.vector._custom_dve(CLIN, out=PL[:, B:4 * B],
                                      in0=cur[:, S0 + B:S0 + 4 * B],
                                      in1=cur[:, S0:S0 + 3 * B],
                                      s0=h_, s1=s_, imm2=2.0 * DX)
                # ghost parts (same ops so phi lands exactly once per point):
                # block 1 L from GL, block 4 R from GR
                nc.vector._custom_dve(CLIN, out=PL[:, 0:B],
                                      in0=cur[:, S0:S0 + B], in1=GLc,
                                      s0=h_, s1=s_, imm2=2.0 * DX)
                nc.vector._custom_dve(DLIN, out=PR[:, 3 * B:4 * B],
                                      in0=cur[:, S0 + 3 * B:S0 + 4 * B],
                                      in1=GRc, s0=h_, s1=s_, imm2=-4.0)
                nc.vector.tensor_tensor(S2[:], S1[:], PL[:], OP.add)
                nc.vector.tensor_tensor(TnV, S2[:], PR[:], OP.add)

                # interface blend into b4: SEL in-place, full partitions
                # (ifC = 1 except p63=0; I1 nonzero only at p63)
                b4v = nxt[:, S0 + 3 * B:S0 + 4 * B]
                nc.vector._custom_dve(SEL, out=b4v, in0=b4v, in1=I1[:],
                                      s0=ifC, s1=one_)

                # row boundary (Neumann) with per-core dynamic source row
                # (middle cores self-copy).  Blocks 1-3 go on ACT right
                # after TnV; block 4 (which must wait for the interface
                # blend) goes on Vector so the ACT queue is never stalled.
                nx4 = nxt[:, S0:S0 + 4 * B].rearrange("p (b i) -> p b i", b=4)
                nc.scalar.copy(nx4[:, 0:3, K:K + 1],
                               nx4[:, 0:3, bass.ds(sv_rtop, 1)])
                nc.scalar.copy(nx4[:, 0:3, K + 63:K + 64],
                               nx4[:, 0:3, bass.ds(sv_rbot, 1)])

                # col 511 (p127) Neumann: SEL b4 <- cE*b4 + cD*b3 in-place,
                # full partitions.  Reads b3 rows post-row-copy; the block-4
                # row copies afterwards are idempotent at p127.
                nc.vector._custom_dve(
                    SEL, out=b4v, in0=b4v,
                    in1=nxt[:, S0 + 2 * B:S0 + 3 * B], s0=cE, s1=cD)

                # column boundary: col 0 (p0) on ACT, then block-4 row
                # copies (must follow the interface/col-511 SELs)
                nc.scalar.copy(nxt[0:1, S0:S0 + B], nxt[0:1, S0 + B:S0 + 2 * B])
                nc.scalar.copy(nx4[:, 3:4, K:K + 1],
                               nx4[:, 3:4, bass.ds(sv_rtop, 1)])
                nc.scalar.copy(nx4[:, 3:4, K + 63:K + 64],
                               nx4[:, 3:4, bass.ds(sv_rbot, 1)])

                # ghost row sync (blocking; gpsimd queue keeps Sync free).
                # Bands carry only the 4 state blocks; GL/GR are rebuilt by
                # the ghost-column matmuls placed AFTER the merge below.
                if t in nsync:
                    cc_out = cc_outs[t]
                    nc.gpsimd.dma_start(cc_in[0], nx4[:, :, K:2 * K])
                    nc.gpsimd.dma_start(cc_in[1], nx4[:, :, 64:64 + K])
                    nc.gpsimd.collective_compute(
                        "AllGather", OP.bypass,
                        replica_groups=[list(range(NCORES))],
                        ins=[cc_in[:]], outs=[cc_out[:]])
                    nc.gpsimd.dma_start(nx4[:, :, 0:K],
                                        cc_out[bass.ds(sv_prev, 128)])
                    nc.gpsimd.dma_start(nx4[:, :, 64 + K:64 + 2 * K],
                                        cc_out[bass.ds(sv_next, 128)])

                # ghost column refresh via partition-shift matmuls (full
                # rows).  At sync steps this reads the merged tile, so the
                # new GL/GR include fresh ghost-row values.
                psR = pp.tile([128, B], f32, tag="psR")
                psL = pp.tile([128, B], f32, tag="psL")
                nc.tensor.matmul(psR[:], wr[:], nxt[:, S0:S0 + B],
                                 start=True, stop=True)
                nc.tensor.matmul(psL[:], wl[:], nxt[:, S0 + 3 * B:S0 + 4 * B],
                                 start=True, stop=True)
                nc.scalar.copy(nxt[:, 5 * B + 2:6 * B + 2], psR[:])
                nc.scalar.copy(nxt[:, 0:B], psL[:])

                # output: owned rows (the read has two steps of slack)
                nc.sync.dma_start(out_d[t - 1], nx4[:, :, K:K + 64])

                cur, nxt = nxt, cur
    return nc


def _ensure_ntff_hook():
    """Provide antenv.axon_hooks (missing in this image) so bass_utils can
    NTFF-profile under axon."""
    import sys
    import types
    try:
        from antenv.axon_hooks import get_axon_ntff_profile_hook  # noqa: F401
        return
    except ImportError:
        pass
    mod = types.ModuleType("antenv.axon_hooks")
    mod._hook = None

    def set_axon_ntff_profile_hook(h):
        mod._hook = h

    def get_axon_ntff_profile_hook():
        return mod._hook

    mod.set_axon_ntff_profile_hook = set_axon_ntff_profile_hook
    mod.get_axon_ntff_profile_hook = get_axon_ntff_profile_hook
    sys.modules["antenv.axon_hooks"] = mod
    import antenv
    antenv.axon_hooks = mod
    try:
        from trn_agent_boot.trn_boot import _ntff_profile_via_ctypes
        hook = _ntff_profile_via_ctypes("/opt/axon/libaxon_pjrt.so")
        if hook is not None:
            mod._hook = hook
    except Exception:
        pass


def kernel(u0, k1, k2, alpha1, alpha2):
    global LAST_EXEC_NS, LAST_RESULT
    import concourse.bacc as bacc
    import concourse.bass as bass
    import concourse.tile as tile
    import concourse.mybir as mybir
    from concourse.bass_utils import run_bass_kernel_spmd

    u0 = np.asarray(u0, dtype=np.float32)
    k1f = float(np.asarray(k1).reshape(-1)[0])
    k2f = float(np.asarray(k2).reshape(-1)[0])
    a1f = float(np.asarray(alpha1).reshape(-1)[0])
    a2f = float(np.asarray(alpha2).reshape(-1)[0])

    dx2 = DX * DX
    scal = {"ca": k1f / (k1f + k2f), "cb": k2f / (k1f + k2f)}

    nc = bacc.Bacc(
        "TRN2", target_bir_lowering=False, debug=False,
        num_devices=NCORES,
    )
    _build(nc, tile, mybir, bass, scal)
    nc.compile()

    left = np.arange(128) < 64
    s = np.where(left, DT * a1f / dx2, DT * a2f / dx2).astype(np.float32)
    h = np.where(left, DT * k1f / (2 * DX), DT * k2f / (2 * DX)).astype(np.float32)
    g = np.where(left, DT * k1f, DT * k2f).astype(np.float32)
    WR = np.eye(128, k=-1, dtype=np.float32)   # out[m] = in[m+1]
    WL = np.eye(128, k=+1, dtype=np.float32)   # out[m] = in[m-1]

    m63 = (np.arange(128) == 63).astype(np.float32)
    m127 = (np.arange(128) == 127).astype(np.float32)
    in_maps = []
    for c in range(NCORES):
        ppc = np.zeros((128, 16), np.float32)
        ppc[:, 0] = g
        ppc[:, 1] = -g
        ppc[:, 2] = 1.0 - 4.0 * s + g
        ppc[:, 3] = s
        ppc[:, 4] = h
        ppc[:, 9] = m63 * scal["ca"]       # ifA
        ppc[:, 10] = m63 * scal["cb"]      # ifB
        ppc[:, 11] = 1.0 - m63             # ifC
        ppc[:, 12] = m127                  # cD
        ppc[:, 13] = 1.0 - m127            # cE
        ppc[:, 14] = 1.0                   # ones (SEL s1 for interface)
        prev_off = (2 * (c - 1) + 1) * 128 if c > 0 else 0
        next_off = (2 * (c + 1)) * 128 if c < NCORES - 1 else 0
        rtop = K + 1 if c == 0 else K
        rbot = K + 62 if c == NCORES - 1 else K + 63
        in_maps.append({
            "u0s": _pack_core(u0, c),
            "ppc": ppc,
            "wr": WR,
            "wl": WL,
            "nbrs": np.array([[prev_off, next_off]], dtype=np.uint32),
            "rsel": np.array([[rtop, rbot]], dtype=np.uint32),
            "msk": np.concatenate([
                np.broadcast_to((m63 * scal["ca"])[:, None], (128, B)),
                np.broadcast_to((m63 * scal["cb"])[:, None], (128, B)),
            ], axis=1).astype(np.float32).copy(),
        })

    trace = os.environ.get("ADR_TRACE", "0") == "1"
    if trace:
        _ensure_ntff_hook()
    res = run_bass_kernel_spmd(
        nc, in_maps, core_ids=list(range(NCORES)), trace=trace)
    LAST_EXEC_NS = res.exec_time_ns
    LAST_RESULT = res

    full = np.zeros((NSTEPS, N, N), np.float32)
    for c in range(NCORES):
        arr = np.asarray(res.results[c]["out"]).reshape(NSTEPS, 128, 4, 64)
        full[:, 64 * c:64 * (c + 1), :] = (
            arr.transpose(0, 3, 1, 2).reshape(NSTEPS, 64, 512))
    return full


# revision 55
# speedup vs baseline: 1.2104x; 1.0123x over previous
"""Trainium2 Bass kernel for nn_AdvectionDiffusionReaction2M (v3).

Advection-diffusion-reaction on a 512x512 grid, 199 sequential steps, output =
all intermediate states (199,512,512) f32.

Sharding: rows split 8 ways (64 rows/core) with 16-row ghost zones refreshed
by an AllGather every 16 steps.  SBUF layout per core: flat [128, 6B+2] f32
per state buffer:
    [ GL (B) | pad | b1 b2 b3 b4 (4B) | pad | GR (B) ]
partition p = column group (cols 4p..4p+3 at blocks b1..b4), GL/GR = ghost
columns 4p-1 / 4p+4, i = stored row (96 = 16 ghost + 64 + 16 ghost).  The two
pad columns make the Up/Dn offset views disjoint from GL/GR, so the ghost
column refresh (PE partition-shift matmuls + PSUM->SBUF copies) overlaps the
next step's leading DVE ops instead of serializing the whole step.

The update is regrouped per neighbor with Tc-dependent coefficients
   Tn = Up*(s+h*Tc^2) + Dn*(s-h*Tc^2) + L*(s-h*Tc) + R*(s+h*Tc) + phi(Tc)
   phi = Tc + g*(Tc^3-Tc^2+Tc),  g = h*2dx
computed by fused custom DVE ops (block-edge rows are sacrificial ghost rows,
so row-crossing garbage in Up/Dn is harmless).  L and R are split into an
interior part (blocks) and a one-block ghost part (GL/GR) per pass.
"""

import os
import numpy as np

N = 512
DX = 1.0 / (N - 1)
DT = 1e-7
MB = 256
NCORES = 8
K = 16                      # ghost depth (rows)
RS = 64 + 2 * K             # stored rows per core (96)
NSTEPS = int(os.environ.get("ADR_NSTEPS", "199"))
B = RS                      # block stride in flat free dim
F = 6 * B + 2               # flat state width: GL|pad|b1..b4|pad|GR
S0 = B + 1                  # flat offset of block 1 (state region start)

LAST_EXEC_NS = None
LAST_RESULT = None

_OPS_REGISTERED = {}


def _register_ops():
    """Register custom DVE ops (runtime registration into dve_ops.OPS)."""
    if _OPS_REGISTERED:
        return _OPS_REGISTERED
    import concourse.dve_ops as dve_ops
    from concourse.dve_ops import DveOp, OPS
    from concourse.dve_spec import Spec, Src0, Src1, C0, C1, C2, One, sq, lower
    from concourse.dve_uop import DveOpSpec

    def make_op(name, body, reference):
        for op in OPS:
            if op.name == name:
                return op
        spec = Spec(body=body, reference=reference)
        shas = {}
        for ver in ("v3", "v4"):
            uops = lower(spec, ver=ver)
            tmp = DveOpSpec(name=name, opcode=0, uops=uops, rd1_en=True)
            shas[ver] = tmp.sha(ver)
        op = DveOp(name, spec, subdim=False, uops_sha=shas)
        OPS.append(op)
        dve_ops._SUB_OPCODE_FOR_NAME[name] = (
            dve_ops._CUSTOM_DVE_ROW_BASE + len(OPS) - 1)
        assert dve_ops._SUB_OPCODE_FOR_NAME[name] < 0x20, "opcode row overflow"
        dve_ops.CUSTOM_DVE_SPECS[name] = spec
        return op

    q = sq(Src0)
    gc = C0 * C2                          # g = h * 2dx (hoisted mult)
    # out = Up*(s + h*Tc^2) + g*(Tc^2 - Tc)*Tc      [phi part 1: g(Tc^3-Tc^2)]
    _OPS_REGISTERED["APHI"] = make_op(
        "ADR_APHI",
        Src1 * (C1 + q * C0) + (q - Src0) * gc * Src0,
        lambda in0, in1, s0, s1, imm2:
            in1 * (s1 + in0**2 * s0)
            + (in0**2 - in0) * (s0 * imm2) * in0)
    # out = Dn*(s - h*Tc^2)
    _OPS_REGISTERED["BSQ"] = make_op(
        "ADR_BSQ", Src1 * (C1 - q * C0),
        lambda in0, in1, s0, s1: in1 * (s1 - in0**2 * s0))
    # out = L*(s - h*Tc) + (h*Tc)*2dx               [phi part 3: g*Tc]
    _a = Src0 * C0
    _OPS_REGISTERED["CLIN"] = make_op(
        "ADR_CLIN", Src1 * (C1 - _a) + _a * C2,
        lambda in0, in1, s0, s1, imm2:
            in1 * (s1 - in0 * s0) + in0 * s0 * imm2)
    # out = R*(s + h*Tc) + Tc + (-4)*s*Tc           [phi part 2: (1-4s)Tc]
    _OPS_REGISTERED["DLIN"] = make_op(
        "ADR_DLIN", Src1 * (C1 + _a) + Src0 + Src0 * C1 * C2,
        lambda in0, in1, s0, s1, imm2:
            in1 * (s1 + in0 * s0) + in0 + in0 * s1 * imm2)
    # out = Src0*C0 + Src1*C1  (masked blend / select)
    _OPS_REGISTERED["SEL"] = make_op(
        "ADR_SEL", Src0 * C0 + Src1 * C1,
        lambda in0, in1, s0, s1: in0 * s0 + in1 * s1)
    return _OPS_REGISTERED


def _pack_core(G, c):
    """Full grid (512,512) -> per-core flat tile [128, F] (f32, zero padded).

    Layout per partition p: [GL | 0 | b1 b2 b3 b4 | 0 | GR] where block bj
    holds column 4p+j-1 over the RS stored rows and GL/GR hold cols 4p-1 /
    4p+4.
    """
    lo = 64 * c - K
    S = np.zeros((RS, N), np.float32)
    g0, g1 = max(lo, 0), min(lo + RS, N)
    S[g0 - lo: g1 - lo] = G[g0:g1]
    cols = (4 * np.arange(128)[:, None] - 1 + np.arange(6)[None, :])  # [128,6]
    valid = (cols >= 0) & (cols < N)
    t = S.T[np.clip(cols, 0, N - 1)]          # [128, 6, RS]
    t[~valid] = 0.0
    flat = np.zeros((128, F), np.float32)
    flat[:, 0:B] = t[:, 0]                      # GL
    flat[:, S0:S0 + 4 * B] = t[:, 1:5].reshape(128, 4 * B)
    flat[:, 5 * B + 2:6 * B + 2] = t[:, 5]      # GR
    return np.ascontiguousarray(flat, dtype=np.float32)


def _build(nc, tile, mybir, bass, scal):
    f32 = mybir.dt.float32
    u32 = mybir.dt.uint32
    OP = mybir.AluOpType
    ops = _register_ops()
    APHI, BSQ, CLIN, DLIN, SEL = (ops[k] for k in
                                  ("APHI", "BSQ", "CLIN", "DLIN", "SEL"))

    bf16 = mybir.dt.bfloat16
    BF = bf16 if os.environ.get("ADR_BF16", "1") == "1" else f32
    AF = mybir.ActivationFunctionType
    u0s_d = nc.dram_tensor("u0s", [128, F], f32, kind="ExternalInput").ap()
    ppc_d = nc.dram_tensor("ppc", [128, 16], f32, kind="ExternalInput").ap()
    wr_d = nc.dram_tensor("wr", [128, 128], f32, kind="ExternalInput").ap()
    wl_d = nc.dram_tensor("wl", [128, 128], f32, kind="ExternalInput").ap()
    nbrs_d = nc.dram_tensor("nbrs", [1, 2], u32, kind="ExternalInput").ap()
    rsel_d = nc.dram_tensor("rsel", [1, 2], u32, kind="ExternalInput").ap()
    msk_d = nc.dram_tensor("msk", [128, 2 * B], f32, kind="ExternalInput").ap()
    out_d = nc.dram_tensor("out", [NSTEPS, 128, 4, 64], f32,
                           kind="ExternalOutput").ap()

    ca, cb = scal["ca"], scal["cb"]

    # ghost sync every K steps (synchronous: state-t bands must merge into the
    # state-t tile before step t+1 — any lag breaks time-consistency)
    nsync = [t for t in range(K, NSTEPS, K)]

    with tile.TileContext(nc) as tc:
        with tc.tile_pool(name="state", bufs=1) as sp, \
             tc.tile_pool(name="tmp", bufs=2) as tp, \
             tc.tile_pool(name="psum", bufs=2, space="PSUM") as pp, \
             tc.tile_pool(name="dram", bufs=1, space="DRAM") as dp:

            tA = sp.tile([128, F], f32, tag="tA")
            tB = sp.tile([128, F], f32, tag="tB")
            tC = sp.tile([128, F], f32, tag="tC")
            ppc = sp.tile([128, 16], f32, tag="ppc")
            msk = sp.tile([128, 2 * B], f32, tag="msk")
            wr = sp.tile([128, 128], f32, tag="wr")
            wl = sp.tile([128, 128], f32, tag="wl")

            cc_in = dp.tile([2, 128, 4, K], f32, tag="ccin")
            cc_outs = {t: dp.tile([16 * 128, 4, K], f32, tag=f"ccout{t}",
                                  name=f"ccout{t}", addr_space="Shared")
                       for t in nsync}


            nc.sync.dma_start(tA[:], u0s_d[:])
            nc.sync.dma_start(ppc[:], ppc_d[:])
            nc.sync.dma_start(msk[:], msk_d[:])
            nc.sync.dma_start(wr[:], wr_d[:])
            nc.sync.dma_start(wl[:], wl_d[:])

            rp = nc.alloc_registers("rprev")
            nc.regs_load(rp, nbrs_d[0:1, 0:1])
            sv_prev = nc.snap(rp, min_val=0, max_val=15 * 128)
            rn = nc.alloc_registers("rnext")
            nc.regs_load(rn, nbrs_d[0:1, 1:2])
            sv_next = nc.snap(rn, min_val=0, max_val=15 * 128)
            # per-core Neumann source rows (core 0: 17 else 16; core 7: 78 else 79)
            rt0 = nc.alloc_registers("rtop")
            nc.regs_load(rt0, rsel_d[0:1, 0:1])
            sv_rtop = nc.snap(rt0, min_val=K, max_val=K + 1)
            rb0 = nc.alloc_registers("rbot")
            nc.regs_load(rb0, rsel_d[0:1, 1:2])
            sv_rbot = nc.snap(rb0, min_val=K + 62, max_val=K + 63)

            s_ = ppc[:, 3:4]; h_ = ppc[:, 4:5]
            ifA = ppc[:, 9:10]; ifB = ppc[:, 10:11]; ifC = ppc[:, 11:12]
            cD = ppc[:, 12:13]; cE = ppc[:, 13:14]
            one_ = ppc[:, 14:15]

            # warm-up AllGather (tiny payload): pays the CC cold-start cost
            # while the first steps compute, so the step-16 collective runs
            # closer to warm latency.
            warm_in = dp.tile([1, 64], f32, tag="warmin")
            warm_out = dp.tile([8, 64], f32, tag="warmout", name="warmout",
                               addr_space="Shared")
            nc.gpsimd.collective_compute(
                "AllGather", OP.bypass,
                replica_groups=[list(range(NCORES))],
                ins=[warm_in[:]], outs=[warm_out[:]])

            # triple-buffered state: the output DMA of step t reads buffer
            # written at t; with only two buffers step t+2's TnV hits a WAR
            # wait on that DMA (~0.5us/step observed).  Three buffers give
            # it two extra steps of slack.
            bufs3 = [tA, tB, tC]
            # ghost refresh for the initial state (u0 pack already provides
            # valid GL/GR, so nothing needed before step 1)
            for t in range(1, NSTEPS + 1):
                cur = bufs3[(t - 1) % 3]
                nxt = bufs3[t % 3]
                # views into the flat state
                Tc = cur[:, S0:S0 + 4 * B]
                Up = cur[:, S0 - 1:S0 + 4 * B - 1]
                Dn = cur[:, S0 + 1:S0 + 4 * B + 1]
                GLc = cur[:, 0:B]
                GRc = cur[:, 5 * B + 2:6 * B + 2]
                TnV = nxt[:, S0:S0 + 4 * B]

                PU = tp.tile([128, 4 * B], BF, tag="PU")
                PD = tp.tile([128, 4 * B], BF, tag="PD")
                PL = tp.tile([128, 4 * B], BF, tag="PL")
                PR = tp.tile([128, 4 * B], f32, tag="PR")
                S1 = tp.tile([128, 4 * B], BF, tag="S1")
                S2 = tp.tile([128, 4 * B], BF, tag="S2")
                I1 = tp.tile([128, B], f32, tag="I1")

                # interface precompute (reads OLD state only): one SEL
                # custom op at partition base 0 (custom scalar APs only work
                # at base 0): I1 = ifA*GRc + ifB*b3 (nonzero only at p63)
                nc.vector._custom_dve(
                    SEL, out=I1[:], in0=GRc,
                    in1=cur[:, S0 + 2 * B:S0 + 3 * B], s0=ifA, s1=ifB)

                # neighbor-grouped fused passes (DVE) + bf16 sum tree.
                # Ghost-column parts (CG/DG) are scheduled late so the ghost
                # refresh of the previous step can overlap the leading ops.
                nc.vector._custom_dve(APHI, out=PU[:], in0=Tc, in1=Up,
                                      s0=h_, s1=s_, imm2=2.0 * DX)
                nc.vector._custom_dve(BSQ, out=PD[:], in0=Tc, in1=Dn,
                                      s0=h_, s1=s_)
                # DLIN over blocks 1-3 (R = blocks 2-4), carries phi
                nc.vector._custom_dve(DLIN, out=PR[:, 0:3 * B],
                                      in0=cur[:, S0:S0 + 3 * B],
                                      in1=cur[:, S0 + B:S0 + 4 * B],
                                      s0=h_, s1=s_, imm2=-4.0)
                nc.vector.tensor_tensor(S1[:], PU[:], PD[:], OP.add)
                # CLIN over blocks 2-4 (L = blocks 1-3), carries phi part 3
                nc.vector._custom_dve(CLIN, out=PL[:, B:4 * B],
                                      in0=cur[:, S0 + B:S0 + 4 * B],
                                      in1=cur[:, S0:S0 + 3 * B],
                                      s0=h_, s1=s_, imm2=2.0 * DX)
                # ghost parts (same ops so phi lands exactly once per point):
                # block 1 L from GL, block 4 R from GR
                nc.vector._custom_dve(CLIN, out=PL[:, 0:B],
                                      in0=cur[:, S0:S0 + B], in1=GLc,
                                      s0=h_, s1=s_, imm2=2.0 * DX)
                nc.vector._custom_dve(DLIN, out=PR[:, 3 * B:4 * B],
                                      in0=cur[:, S0 + 3 * B:S0 + 4 * B],
                                      in1=GRc, s0=h_, s1=s_, imm2=-4.0)
                nc.vector.tensor_tensor(S2[:], S1[:], PL[:], OP.add)
                nc.vector.tensor_tensor(TnV, S2[:], PR[:], OP.add)

                # interface blend into b4: SEL in-place, full partitions
                # (ifC = 1 except p63=0; I1 nonzero only at p63)
                b4v = nxt[:, S0 + 3 * B:S0 + 4 * B]
                nc.vector._custom_dve(SEL, out=b4v, in0=b4v, in1=I1[:],
                                      s0=ifC, s1=one_)

                # row boundary (Neumann) with per-core dynamic source row
                # (middle cores self-copy).  Blocks 1-3 go on ACT right
                # after TnV; block 4 (which must wait for the interface
                # blend) goes on Vector so the ACT queue is never stalled.
                nx4 = nxt[:, S0:S0 + 4 * B].rearrange("p (b i) -> p b i", b=4)
                nc.scalar.copy(nx4[:, 0:3, K:K + 1],
                               nx4[:, 0:3, bass.ds(sv_rtop, 1)])
                nc.scalar.copy(nx4[:, 0:3, K + 63:K + 64],
                               nx4[:, 0:3, bass.ds(sv_rbot, 1)])

                # col 511 (p127) Neumann: SEL b4 <- cE*b4 + cD*b3 in-place,
                # full partitions.  Reads b3 rows post-row-copy; the block-4
                # row copies afterwards are idempotent at p127.
                nc.vector._custom_dve(
                    SEL, out=b4v, in0=b4v,
                    in1=nxt[:, S0 + 2 * B:S0 + 3 * B], s0=cE, s1=cD)

                # column boundary: col 0 (p0) on ACT, then block-4 row
                # copies (must follow the interface/col-511 SELs)
                nc.scalar.copy(nxt[0:1, S0:S0 + B], nxt[0:1, S0 + B:S0 + 2 * B])
                nc.scalar.copy(nx4[:, 3:4, K:K + 1],
                               nx4[:, 3:4, bass.ds(sv_rtop, 1)])
                nc.scalar.copy(nx4[:, 3:4, K + 63:K + 64],
                               nx4[:, 3:4, bass.ds(sv_rbot, 1)])

                # ghost row sync (blocking; gpsimd queue keeps Sync free).
                # Bands carry only the 4 state blocks; GL/GR are rebuilt by
                # the ghost-column matmuls placed AFTER the merge below.
                if t in nsync:
                    cc_out = cc_outs[t]
                    nc.gpsimd.dma_start(cc_in[0], nx4[:, :, K:2 * K])
                    nc.gpsimd.dma_start(cc_in[1], nx4[:, :, 64:64 + K])
                    nc.gpsimd.collective_compute(
                        "AllGather", OP.bypass,
                        replica_groups=[list(range(NCORES))],
                        ins=[cc_in[:]], outs=[cc_out[:]])
                    nc.gpsimd.dma_start(nx4[:, :, 0:K],
                                        cc_out[bass.ds(sv_prev, 128)])
                    nc.gpsimd.dma_start(nx4[:, :, 64 + K:64 + 2 * K],
                                        cc_out[bass.ds(sv_next, 128)])

                # ghost column refresh via partition-shift matmuls (full
                # rows).  At sync steps this reads the merged tile, so the
                # new GL/GR include fresh ghost-row values.
                psR = pp.tile([128, B], f32, tag="psR")
                psL = pp.tile([128, B], f32, tag="psL")
                nc.tensor.matmul(psR[:], wr[:], nxt[:, S0:S0 + B],
                                 start=True, stop=True)
                nc.tensor.matmul(psL[:], wl[:], nxt[:, S0 + 3 * B:S0 + 4 * B],
                                 start=True, stop=True)
                nc.scalar.copy(nxt[:, 5 * B + 2:6 * B + 2], psR[:])
                nc.scalar.copy(nxt[:, 0:B], psL[:])

                # output: owned rows (the read has three steps of slack)
                nc.sync.dma_start(out_d[t - 1], nx4[:, :, K:K + 64])
    return nc


def _ensure_ntff_hook():
    """Provide antenv.axon_hooks (missing in this image) so bass_utils can
    NTFF-profile under axon."""
    import sys
    import types
    try:
        from antenv.axon_hooks import get_axon_ntff_profile_hook  # noqa: F401
        return
    except ImportError:
        pass
    mod = types.ModuleType("antenv.axon_hooks")
    mod._hook = None

    def set_axon_ntff_profile_hook(h):
        mod._hook = h

    def get_axon_ntff_profile_hook():
        return mod._hook

    mod.set_axon_ntff_profile_hook = set_axon_ntff_profile_hook
    mod.get_axon_ntff_profile_hook = get_axon_ntff_profile_hook
    sys.modules["antenv.axon_hooks"] = mod
    import antenv
    antenv.axon_hooks = mod
    try:
        from trn_agent_boot.trn_boot import _ntff_profile_via_ctypes
        hook = _ntff_profile_via_ctypes("/opt/axon/libaxon_pjrt.so")
        if hook is not None:
            mod._hook = hook
    except Exception:
        pass


def kernel(u0, k1, k2, alpha1, alpha2):
    global LAST_EXEC_NS, LAST_RESULT
    import concourse.bacc as bacc
    import concourse.bass as bass
    import concourse.tile as tile
    import concourse.mybir as mybir
    from concourse.bass_utils import run_bass_kernel_spmd

    u0 = np.asarray(u0, dtype=np.float32)
    k1f = float(np.asarray(k1).reshape(-1)[0])
    k2f = float(np.asarray(k2).reshape(-1)[0])
    a1f = float(np.asarray(alpha1).reshape(-1)[0])
    a2f = float(np.asarray(alpha2).reshape(-1)[0])

    dx2 = DX * DX
    scal = {"ca": k1f / (k1f + k2f), "cb": k2f / (k1f + k2f)}

    nc = bacc.Bacc(
        "TRN2", target_bir_lowering=False, debug=False,
        num_devices=NCORES,
    )
    _build(nc, tile, mybir, bass, scal)
    nc.compile()

    left = np.arange(128) < 64
    s = np.where(left, DT * a1f / dx2, DT * a2f / dx2).astype(np.float32)
    h = np.where(left, DT * k1f / (2 * DX), DT * k2f / (2 * DX)).astype(np.float32)
    g = np.where(left, DT * k1f, DT * k2f).astype(np.float32)
    WR = np.eye(128, k=-1, dtype=np.float32)   # out[m] = in[m+1]
    WL = np.eye(128, k=+1, dtype=np.float32)   # out[m] = in[m-1]

    m63 = (np.arange(128) == 63).astype(np.float32)
    m127 = (np.arange(128) == 127).astype(np.float32)
    in_maps = []
    for c in range(NCORES):
        ppc = np.zeros((128, 16), np.float32)
        ppc[:, 0] = g
        ppc[:, 1] = -g
        ppc[:, 2] = 1.0 - 4.0 * s + g
        ppc[:, 3] = s
        ppc[:, 4] = h
        ppc[:, 9] = m63 * scal["ca"]       # ifA
        ppc[:, 10] = m63 * scal["cb"]      # ifB
        ppc[:, 11] = 1.0 - m63             # ifC
        ppc[:, 12] = m127                  # cD
        ppc[:, 13] = 1.0 - m127            # cE
        ppc[:, 14] = 1.0                   # ones (SEL s1 for interface)
        prev_off = (2 * (c - 1) + 1) * 128 if c > 0 else 0
        next_off = (2 * (c + 1)) * 128 if c < NCORES - 1 else 0
        rtop = K + 1 if c == 0 else K
        rbot = K + 62 if c == NCORES - 1 else K + 63
        in_maps.append({
            "u0s": _pack_core(u0, c),
            "ppc": ppc,
            "wr": WR,
            "wl": WL,
            "nbrs": np.array([[prev_off, next_off]], dtype=np.uint32),
            "rsel": np.array([[rtop, rbot]], dtype=np.uint32),
            "msk": np.concatenate([
                np.broadcast_to((m63 * scal["ca"])[:, None], (128, B)),
                np.broadcast_to((m63 * scal["cb"])[:, None], (128, B)),
            ], axis=1).astype(np.float32).copy(),
        })

    trace = os.environ.get("ADR_TRACE", "0") == "1"
    if trace:
        _ensure_ntff_hook()
    res = run_bass_kernel_spmd(
        nc, in_maps, core_ids=list(range(NCORES)), trace=trace)
    LAST_EXEC_NS = res.exec_time_ns
    LAST_RESULT = res

    full = np.zeros((NSTEPS, N, N), np.float32)
    for c in range(NCORES):
        arr = np.asarray(res.results[c]["out"]).reshape(NSTEPS, 128, 4, 64)
        full[:, 64 * c:64 * (c + 1), :] = (
            arr.transpose(0, 3, 1, 2).reshape(NSTEPS, 64, 512))
    return full


# revision 63
# speedup vs baseline: 1.2312x; 1.0171x over previous
"""Trainium2 Bass kernel for nn_AdvectionDiffusionReaction2M (v3).

Advection-diffusion-reaction on a 512x512 grid, 199 sequential steps, output =
all intermediate states (199,512,512) f32.

Sharding: rows split 8 ways (64 rows/core) with 16-row ghost zones refreshed
by an AllGather every 16 steps.  SBUF layout per core: flat [128, 6B+2] f32
per state buffer:
    [ GL (B) | pad | b1 b2 b3 b4 (4B) | pad | GR (B) ]
partition p = column group (cols 4p..4p+3 at blocks b1..b4), GL/GR = ghost
columns 4p-1 / 4p+4, i = stored row (96 = 16 ghost + 64 + 16 ghost).  The two
pad columns make the Up/Dn offset views disjoint from GL/GR, so the ghost
column refresh (PE partition-shift matmuls + PSUM->SBUF copies) overlaps the
next step's leading DVE ops instead of serializing the whole step.

The update is regrouped per neighbor with Tc-dependent coefficients
   Tn = Up*(s+h*Tc^2) + Dn*(s-h*Tc^2) + L*(s-h*Tc) + R*(s+h*Tc) + phi(Tc)
   phi = Tc + g*(Tc^3-Tc^2+Tc),  g = h*2dx
computed by fused custom DVE ops (block-edge rows are sacrificial ghost rows,
so row-crossing garbage in Up/Dn is harmless).  L and R are split into an
interior part (blocks) and a one-block ghost part (GL/GR) per pass.
"""

import os
import numpy as np

N = 512
DX = 1.0 / (N - 1)
DT = 1e-7
MB = 256
NCORES = 8
K = 16                      # ghost depth (rows)
RS = 64 + 2 * K             # stored rows per core (96)
NSTEPS = int(os.environ.get("ADR_NSTEPS", "199"))
B = RS                      # block stride in flat free dim
F = 6 * B + 2               # flat state width: GL|pad|b1..b4|pad|GR
S0 = B + 1                  # flat offset of block 1 (state region start)

LAST_EXEC_NS = None
LAST_RESULT = None

_OPS_REGISTERED = {}


def _register_ops():
    """Register custom DVE ops (runtime registration into dve_ops.OPS)."""
    if _OPS_REGISTERED:
        return _OPS_REGISTERED
    import concourse.dve_ops as dve_ops
    from concourse.dve_ops import DveOp, OPS
    from concourse.dve_spec import Spec, Src0, Src1, C0, C1, C2, One, sq, lower
    from concourse.dve_uop import DveOpSpec

    def make_op(name, body, reference):
        for op in OPS:
            if op.name == name:
                return op
        spec = Spec(body=body, reference=reference)
        shas = {}
        for ver in ("v3", "v4"):
            uops = lower(spec, ver=ver)
            tmp = DveOpSpec(name=name, opcode=0, uops=uops, rd1_en=True)
            shas[ver] = tmp.sha(ver)
        op = DveOp(name, spec, subdim=False, uops_sha=shas)
        OPS.append(op)
        dve_ops._SUB_OPCODE_FOR_NAME[name] = (
            dve_ops._CUSTOM_DVE_ROW_BASE + len(OPS) - 1)
        assert dve_ops._SUB_OPCODE_FOR_NAME[name] < 0x20, "opcode row overflow"
        dve_ops.CUSTOM_DVE_SPECS[name] = spec
        return op

    q = sq(Src0)
    gc = C0 * C2                          # g = h * 2dx (hoisted mult)
    # out = Up*(s + h*Tc^2) + g*(Tc^2 - Tc)*Tc      [phi part 1: g(Tc^3-Tc^2)]
    _OPS_REGISTERED["APHI"] = make_op(
        "ADR_APHI",
        Src1 * (C1 + q * C0) + (q - Src0) * gc * Src0,
        lambda in0, in1, s0, s1, imm2:
            in1 * (s1 + in0**2 * s0)
            + (in0**2 - in0) * (s0 * imm2) * in0)
    # out = Dn*(s - h*Tc^2)
    _OPS_REGISTERED["BSQ"] = make_op(
        "ADR_BSQ", Src1 * (C1 - q * C0),
        lambda in0, in1, s0, s1: in1 * (s1 - in0**2 * s0))
    # out = L*(s - h*Tc) + (h*Tc)*2dx               [phi part 3: g*Tc]
    _a = Src0 * C0
    _OPS_REGISTERED["CLIN"] = make_op(
        "ADR_CLIN", Src1 * (C1 - _a) + _a * C2,
        lambda in0, in1, s0, s1, imm2:
            in1 * (s1 - in0 * s0) + in0 * s0 * imm2)
    # out = R*(s + h*Tc) + Tc + (-4)*s*Tc           [phi part 2: (1-4s)Tc]
    _OPS_REGISTERED["DLIN"] = make_op(
        "ADR_DLIN", Src1 * (C1 + _a) + Src0 + Src0 * C1 * C2,
        lambda in0, in1, s0, s1, imm2:
            in1 * (s1 + in0 * s0) + in0 + in0 * s1 * imm2)
    # out = Src0*C0 + Src1*C1  (masked blend / select)
    _OPS_REGISTERED["SEL"] = make_op(
        "ADR_SEL", Src0 * C0 + Src1 * C1,
        lambda in0, in1, s0, s1: in0 * s0 + in1 * s1)
    return _OPS_REGISTERED


def _pack_core(G, c):
    """Full grid (512,512) -> per-core flat tile [128, F] (f32, zero padded).

    Layout per partition p: [GL | 0 | b1 b2 b3 b4 | 0 | GR] where block bj
    holds column 4p+j-1 over the RS stored rows and GL/GR hold cols 4p-1 /
    4p+4.
    """
    lo = 64 * c - K
    S = np.zeros((RS, N), np.float32)
    g0, g1 = max(lo, 0), min(lo + RS, N)
    S[g0 - lo: g1 - lo] = G[g0:g1]
    cols = (4 * np.arange(128)[:, None] - 1 + np.arange(6)[None, :])  # [128,6]
    valid = (cols >= 0) & (cols < N)
    t = S.T[np.clip(cols, 0, N - 1)]          # [128, 6, RS]
    t[~valid] = 0.0
    flat = np.zeros((128, F), np.float32)
    flat[:, 0:B] = t[:, 0]                      # GL
    flat[:, S0:S0 + 4 * B] = t[:, 1:5].reshape(128, 4 * B)
    flat[:, 5 * B + 2:6 * B + 2] = t[:, 5]      # GR
    return np.ascontiguousarray(flat, dtype=np.float32)


def _build(nc, tile, mybir, bass, scal):
    f32 = mybir.dt.float32
    u32 = mybir.dt.uint32
    OP = mybir.AluOpType
    ops = _register_ops()
    APHI, BSQ, CLIN, DLIN, SEL = (ops[k] for k in
                                  ("APHI", "BSQ", "CLIN", "DLIN", "SEL"))

    bf16 = mybir.dt.bfloat16
    BF = bf16 if os.environ.get("ADR_BF16", "1") == "1" else f32
    AF = mybir.ActivationFunctionType
    u0s_d = nc.dram_tensor("u0s", [128, F], f32, kind="ExternalInput").ap()
    ppc_d = nc.dram_tensor("ppc", [128, 16], f32, kind="ExternalInput").ap()
    wr_d = nc.dram_tensor("wr", [128, 128], f32, kind="ExternalInput").ap()
    wl_d = nc.dram_tensor("wl", [128, 128], f32, kind="ExternalInput").ap()
    nbrs_d = nc.dram_tensor("nbrs", [1, 2], u32, kind="ExternalInput").ap()
    rsel_d = nc.dram_tensor("rsel", [1, 2], u32, kind="ExternalInput").ap()
    msk_d = nc.dram_tensor("msk", [128, 2 * B], f32, kind="ExternalInput").ap()
    out_d = nc.dram_tensor("out", [NSTEPS, 128, 4, 64], f32,
                           kind="ExternalOutput").ap()

    ca, cb = scal["ca"], scal["cb"]

    # ghost sync every K steps (synchronous: state-t bands must merge into the
    # state-t tile before step t+1 — any lag breaks time-consistency)
    nsync = [t for t in range(K, NSTEPS, K)]

    with tile.TileContext(nc) as tc:
        with tc.tile_pool(name="state", bufs=1) as sp, \
             tc.tile_pool(name="tmp", bufs=2) as tp, \
             tc.tile_pool(name="psum", bufs=2, space="PSUM") as pp, \
             tc.tile_pool(name="dram", bufs=1, space="DRAM") as dp:

            tA = sp.tile([128, F], f32, tag="tA")
            tB = sp.tile([128, F], f32, tag="tB")
            tC = sp.tile([128, F], f32, tag="tC")
            ppc = sp.tile([128, 16], f32, tag="ppc")
            msk = sp.tile([128, 2 * B], f32, tag="msk")
            wr = sp.tile([128, 128], f32, tag="wr")
            wl = sp.tile([128, 128], f32, tag="wl")

            cc_in = dp.tile([2, 128, 4, K], f32, tag="ccin")
            cc_outs = {t: dp.tile([16 * 128, 4, K], f32, tag=f"ccout{t}",
                                  name=f"ccout{t}", addr_space="Shared")
                       for t in nsync}


            nc.sync.dma_start(tA[:], u0s_d[:])
            nc.sync.dma_start(ppc[:], ppc_d[:])
            nc.sync.dma_start(msk[:], msk_d[:])
            nc.sync.dma_start(wr[:], wr_d[:])
            nc.sync.dma_start(wl[:], wl_d[:])

            rp = nc.alloc_registers("rprev")
            nc.regs_load(rp, nbrs_d[0:1, 0:1])
            sv_prev = nc.snap(rp, min_val=0, max_val=15 * 128)
            rn = nc.alloc_registers("rnext")
            nc.regs_load(rn, nbrs_d[0:1, 1:2])
            sv_next = nc.snap(rn, min_val=0, max_val=15 * 128)
            # per-core Neumann source rows (core 0: 17 else 16; core 7: 78 else 79)
            rt0 = nc.alloc_registers("rtop")
            nc.regs_load(rt0, rsel_d[0:1, 0:1])
            sv_rtop = nc.snap(rt0, min_val=K, max_val=K + 1)
            rb0 = nc.alloc_registers("rbot")
            nc.regs_load(rb0, rsel_d[0:1, 1:2])
            sv_rbot = nc.snap(rb0, min_val=K + 62, max_val=K + 63)

            s_ = ppc[:, 3:4]; h_ = ppc[:, 4:5]
            ifA = ppc[:, 9:10]; ifB = ppc[:, 10:11]; ifC = ppc[:, 11:12]
            cD = ppc[:, 12:13]; cE = ppc[:, 13:14]
            one_ = ppc[:, 14:15]

            # warm-up AllGather (tiny payload): pays the CC cold-start cost
            # while the first steps compute, so the step-16 collective runs
            # closer to warm latency.
            warm_in = dp.tile([1, 64], f32, tag="warmin")
            warm_out = dp.tile([8, 64], f32, tag="warmout", name="warmout",
                               addr_space="Shared")
            nc.gpsimd.collective_compute(
                "AllGather", OP.bypass,
                replica_groups=[list(range(NCORES))],
                ins=[warm_in[:]], outs=[warm_out[:]])

            # triple-buffered state: the output DMA of step t reads buffer
            # written at t; with only two buffers step t+2's TnV hits a WAR
            # wait on that DMA (~0.5us/step observed).  Three buffers give
            # it two extra steps of slack.
            bufs3 = [tA, tB, tC]
            # ghost refresh for the initial state (u0 pack already provides
            # valid GL/GR, so nothing needed before step 1)
            for t in range(1, NSTEPS + 1):
                cur = bufs3[(t - 1) % 3]
                nxt = bufs3[t % 3]
                # ghost rows decay one row per step since the last refresh:
                # only rows [lo, hi) need computing this step.  Ops whose
                # encoding allows it use windowed views (ops with imm2
                # can't take 2-free-dim inputs, so APHI/CLIN2/DLIN1 stay
                # full-width).
                m = ((t - 1) % K) + 1
                lo, hi = m, RS - m
                # views into the flat state
                Tc = cur[:, S0:S0 + 4 * B]
                Up = cur[:, S0 - 1:S0 + 4 * B - 1]
                Dn = cur[:, S0 + 1:S0 + 4 * B + 1]
                GLc = cur[:, 0:B]
                GRc = cur[:, 5 * B + 2:6 * B + 2]
                cur3 = cur[:, S0:S0 + 4 * B].rearrange("p (b i) -> p b i",
                                                       b=4)
                nxt3 = nxt[:, S0:S0 + 4 * B].rearrange("p (b i) -> p b i",
                                                       b=4)

                PU = tp.tile([128, 4 * B], BF, tag="PU")
                PD = tp.tile([128, 4 * B], BF, tag="PD")
                PL = tp.tile([128, 4 * B], BF, tag="PL")
                PR = tp.tile([128, 4 * B], f32, tag="PR")
                S1 = tp.tile([128, 4 * B], BF, tag="S1")
                S2 = tp.tile([128, 4 * B], BF, tag="S2")
                I1 = tp.tile([128, B], f32, tag="I1")

                # interface precompute (reads OLD state only): one SEL
                # custom op at partition base 0 (custom scalar APs only work
                # at base 0): I1 = ifA*GRc + ifB*b3 (nonzero only at p63)
                nc.vector._custom_dve(
                    SEL, out=I1[:, lo:hi], in0=GRc[:, lo:hi],
                    in1=cur[:, S0 + 2 * B + lo:S0 + 2 * B + hi],
                    s0=ifA, s1=ifB)

                # neighbor-grouped fused passes (DVE) + bf16 sum tree.
                # Ghost-column parts (CG/DG) are scheduled late so the ghost
                # refresh of the previous step can overlap the leading ops.
                nc.vector._custom_dve(APHI, out=PU[:], in0=Tc, in1=Up,
                                      s0=h_, s1=s_, imm2=2.0 * DX)
                PD3 = PD[:].rearrange("p (b i) -> p b i", b=4)
                nc.vector._custom_dve(BSQ, out=PD[:], in0=Tc, in1=Dn,
                                      s0=h_, s1=s_)
                # DLIN over blocks 1-3 (R = blocks 2-4), carries phi
                nc.vector._custom_dve(DLIN, out=PR[:, 0:3 * B],
                                      in0=cur[:, S0:S0 + 3 * B],
                                      in1=cur[:, S0 + B:S0 + 4 * B],
                                      s0=h_, s1=s_, imm2=-4.0)
                PU3 = PU[:].rearrange("p (b i) -> p b i", b=4)
                S13 = S1[:].rearrange("p (b i) -> p b i", b=4)
                nc.vector.tensor_tensor(S13[:, :, lo:hi], PU3[:, :, lo:hi],
                                        PD3[:, :, lo:hi], OP.add)
                # CLIN over blocks 2-4 (L = blocks 1-3), carries phi part 3
                nc.vector._custom_dve(CLIN, out=PL[:, B:4 * B],
                                      in0=cur[:, S0 + B:S0 + 4 * B],
                                      in1=cur[:, S0:S0 + 3 * B],
                                      s0=h_, s1=s_, imm2=2.0 * DX)
                # ghost parts (same ops so phi lands exactly once per point):
                # block 1 L from GL, block 4 R from GR (windowed: these are
                # single-block 1-free-dim views, so imm2 is allowed)
                nc.vector._custom_dve(CLIN, out=PL[:, lo:hi],
                                      in0=cur[:, S0 + lo:S0 + hi],
                                      in1=GLc[:, lo:hi],
                                      s0=h_, s1=s_, imm2=2.0 * DX)
                nc.vector._custom_dve(DLIN, out=PR[:, 3 * B + lo:3 * B + hi],
                                      in0=cur[:, S0 + 3 * B + lo:S0 + 3 * B + hi],
                                      in1=GRc[:, lo:hi],
                                      s0=h_, s1=s_, imm2=-4.0)
                PL3 = PL[:].rearrange("p (b i) -> p b i", b=4)
                PR3 = PR[:].rearrange("p (b i) -> p b i", b=4)
                S23 = S2[:].rearrange("p (b i) -> p b i", b=4)
                nc.vector.tensor_tensor(S23[:, :, lo:hi], S13[:, :, lo:hi],
                                        PL3[:, :, lo:hi], OP.add)
                nc.vector.tensor_tensor(nxt3[:, :, lo:hi], S23[:, :, lo:hi],
                                        PR3[:, :, lo:hi], OP.add)

                # interface blend into b4: SEL in-place, full partitions
                # (ifC = 1 except p63=0; I1 nonzero only at p63)
                b4v = nxt[:, S0 + 3 * B + lo:S0 + 3 * B + hi]
                nc.vector._custom_dve(SEL, out=b4v, in0=b4v,
                                      in1=I1[:, lo:hi], s0=ifC, s1=one_)

                # row boundary (Neumann) with per-core dynamic source row
                # (middle cores self-copy).  Blocks 1-3 go on ACT right
                # after TnV; block 4 (which must wait for the interface
                # blend) goes on Vector so the ACT queue is never stalled.
                nx4 = nxt[:, S0:S0 + 4 * B].rearrange("p (b i) -> p b i", b=4)
                nc.scalar.copy(nx4[:, 0:3, K:K + 1],
                               nx4[:, 0:3, bass.ds(sv_rtop, 1)])
                nc.scalar.copy(nx4[:, 0:3, K + 63:K + 64],
                               nx4[:, 0:3, bass.ds(sv_rbot, 1)])

                # col 511 (p127) Neumann: SEL b4 <- cE*b4 + cD*b3 in-place,
                # full partitions.  Reads b3 rows post-row-copy; the block-4
                # row copies afterwards are idempotent at p127.
                nc.vector._custom_dve(
                    SEL, out=b4v, in0=b4v,
                    in1=nxt[:, S0 + 2 * B + lo:S0 + 2 * B + hi],
                    s0=cE, s1=cD)

                # column boundary: col 0 (p0) on ACT, then block-4 row
                # copies (must follow the interface/col-511 SELs)
                nc.scalar.copy(nxt[0:1, S0:S0 + B], nxt[0:1, S0 + B:S0 + 2 * B])
                nc.scalar.copy(nx4[:, 3:4, K:K + 1],
                               nx4[:, 3:4, bass.ds(sv_rtop, 1)])
                nc.scalar.copy(nx4[:, 3:4, K + 63:K + 64],
                               nx4[:, 3:4, bass.ds(sv_rbot, 1)])

                # ghost row sync (blocking; gpsimd queue keeps Sync free).
                # Bands carry only the 4 state blocks; GL/GR are rebuilt by
                # the ghost-column matmuls placed AFTER the merge below.
                if t in nsync:
                    cc_out = cc_outs[t]
                    nc.gpsimd.dma_start(cc_in[0], nx4[:, :, K:2 * K])
                    nc.gpsimd.dma_start(cc_in[1], nx4[:, :, 64:64 + K])
                    nc.gpsimd.collective_compute(
                        "AllGather", OP.bypass,
                        replica_groups=[list(range(NCORES))],
                        ins=[cc_in[:]], outs=[cc_out[:]])
                    nc.gpsimd.dma_start(nx4[:, :, 0:K],
                                        cc_out[bass.ds(sv_prev, 128)])
                    nc.gpsimd.dma_start(nx4[:, :, 64 + K:64 + 2 * K],
                                        cc_out[bass.ds(sv_next, 128)])

                # ghost column refresh via partition-shift matmuls (full
                # rows).  At sync steps this reads the merged tile, so the
                # new GL/GR include fresh ghost-row values.
                psR = pp.tile([128, B], f32, tag="psR")
                psL = pp.tile([128, B], f32, tag="psL")
                nc.tensor.matmul(psR[:], wr[:], nxt[:, S0:S0 + B],
                                 start=True, stop=True)
                nc.tensor.matmul(psL[:], wl[:], nxt[:, S0 + 3 * B:S0 + 4 * B],
                                 start=True, stop=True)
                nc.scalar.copy(nxt[:, 5 * B + 2:6 * B + 2], psR[:])
                nc.scalar.copy(nxt[:, 0:B], psL[:])

                # output: owned rows (the read has three steps of slack)
                nc.sync.dma_start(out_d[t - 1], nx4[:, :, K:K + 64])
    return nc


def _ensure_ntff_hook():
    """Provide antenv.axon_hooks (missing in this image) so bass_utils can
    NTFF-profile under axon."""
    import sys
    import types
    try:
        from antenv.axon_hooks import get_axon_ntff_profile_hook  # noqa: F401
        return
    except ImportError:
        pass
    mod = types.ModuleType("antenv.axon_hooks")
    mod._hook = None

    def set_axon_ntff_profile_hook(h):
        mod._hook = h

    def get_axon_ntff_profile_hook():
        return mod._hook

    mod.set_axon_ntff_profile_hook = set_axon_ntff_profile_hook
    mod.get_axon_ntff_profile_hook = get_axon_ntff_profile_hook
    sys.modules["antenv.axon_hooks"] = mod
    import antenv
    antenv.axon_hooks = mod
    try:
        from trn_agent_boot.trn_boot import _ntff_profile_via_ctypes
        hook = _ntff_profile_via_ctypes("/opt/axon/libaxon_pjrt.so")
        if hook is not None:
            mod._hook = hook
    except Exception:
        pass


def kernel(u0, k1, k2, alpha1, alpha2):
    global LAST_EXEC_NS, LAST_RESULT
    import concourse.bacc as bacc
    import concourse.bass as bass
    import concourse.tile as tile
    import concourse.mybir as mybir
    from concourse.bass_utils import run_bass_kernel_spmd

    u0 = np.asarray(u0, dtype=np.float32)
    k1f = float(np.asarray(k1).reshape(-1)[0])
    k2f = float(np.asarray(k2).reshape(-1)[0])
    a1f = float(np.asarray(alpha1).reshape(-1)[0])
    a2f = float(np.asarray(alpha2).reshape(-1)[0])

    dx2 = DX * DX
    scal = {"ca": k1f / (k1f + k2f), "cb": k2f / (k1f + k2f)}

    nc = bacc.Bacc(
        "TRN2", target_bir_lowering=False, debug=False,
        num_devices=NCORES,
    )
    _build(nc, tile, mybir, bass, scal)
    nc.compile()

    left = np.arange(128) < 64
    s = np.where(left, DT * a1f / dx2, DT * a2f / dx2).astype(np.float32)
    h = np.where(left, DT * k1f / (2 * DX), DT * k2f / (2 * DX)).astype(np.float32)
    g = np.where(left, DT * k1f, DT * k2f).astype(np.float32)
    WR = np.eye(128, k=-1, dtype=np.float32)   # out[m] = in[m+1]
    WL = np.eye(128, k=+1, dtype=np.float32)   # out[m] = in[m-1]

    m63 = (np.arange(128) == 63).astype(np.float32)
    m127 = (np.arange(128) == 127).astype(np.float32)
    in_maps = []
    for c in range(NCORES):
        ppc = np.zeros((128, 16), np.float32)
        ppc[:, 0] = g
        ppc[:, 1] = -g
        ppc[:, 2] = 1.0 - 4.0 * s + g
        ppc[:, 3] = s
        ppc[:, 4] = h
        ppc[:, 9] = m63 * scal["ca"]       # ifA
        ppc[:, 10] = m63 * scal["cb"]      # ifB
        ppc[:, 11] = 1.0 - m63             # ifC
        ppc[:, 12] = m127                  # cD
        ppc[:, 13] = 1.0 - m127            # cE
        ppc[:, 14] = 1.0                   # ones (SEL s1 for interface)
        prev_off = (2 * (c - 1) + 1) * 128 if c > 0 else 0
        next_off = (2 * (c + 1)) * 128 if c < NCORES - 1 else 0
        rtop = K + 1 if c == 0 else K
        rbot = K + 62 if c == NCORES - 1 else K + 63
        in_maps.append({
            "u0s": _pack_core(u0, c),
            "ppc": ppc,
            "wr": WR,
            "wl": WL,
            "nbrs": np.array([[prev_off, next_off]], dtype=np.uint32),
            "rsel": np.array([[rtop, rbot]], dtype=np.uint32),
            "msk": np.concatenate([
                np.broadcast_to((m63 * scal["ca"])[:, None], (128, B)),
                np.broadcast_to((m63 * scal["cb"])[:, None], (128, B)),
            ], axis=1).astype(np.float32).copy(),
        })

    trace = os.environ.get("ADR_TRACE", "0") == "1"
    if trace:
        _ensure_ntff_hook()
    res = run_bass_kernel_spmd(
        nc, in_maps, core_ids=list(range(NCORES)), trace=trace)
    LAST_EXEC_NS = res.exec_time_ns
    LAST_RESULT = res

    full = np.zeros((NSTEPS, N, N), np.float32)
    for c in range(NCORES):
        arr = np.asarray(res.results[c]["out"]).reshape(NSTEPS, 128, 4, 64)
        full[:, 64 * c:64 * (c + 1), :] = (
            arr.transpose(0, 3, 1, 2).reshape(NSTEPS, 64, 512))
    return full
